# revision 23
# baseline (speedup 1.0000x reference)
"""GCN (3-layer GraphConv, norm='right') — 8-core SPMD Trainium2 Bass kernel.

Strategy (src-sharded edges + per-layer split ReduceScatter):
  Nodes are split into 8 contiguous shards of 49 blocks (6272 rows). Core c
  owns edges whose SRC lies in its shard, so every gather reads only the
  core-local projected table (single int16 index stream, 256B fp16 rows).
  Per layer, each core aggregates its edges' messages into a FULL 392-block
  partial table (one-hot S-matmuls with inv_deg folded in, fp16), then a
  3-segment ReduceScatter(add) — early segments issued mid-layer — sums
  partials and hands each core its dst shard. The epilogue applies
  bias/relu and the next layer's projection locally; no AllGather is ever
  needed because next-layer gathers only read local rows.

Perf notes (cost-model driven):
  - dma_gather descriptors are 256B (hard API floor) -> gather dominates DMA.
  - partial tables live in per-(segment) slab tensors [8, 128, n_g*64] so the
    staged write is one fat-descriptor DMA per (segment, core) and the RS
    piece for core c is exactly slab c; agg comes back partition-major.
  - PSUM accumulates 8 dst blocks per [128, 512] bank; one ACT activation
    evacuates all 8 (amortizes the ~185ns ACT init).
  - tables store real data in cols 0:64 of 128-wide rows; only those 128B
    are ever written (gathers read the full 256B row, upper half junk).
  - S-builds: ~1/10 on Pool (gpsimd), rest on DVE; Pool also runs gather
    desc-gen (994ns fixed per call -> TPC=48 tiles per gather).
"""
import numpy as np
import ml_dtypes

import concourse.bass as bass
import concourse.tile as tile
from concourse import bacc, mybir
from concourse.bass_utils import run_bass_kernel_spmd

BF = np.float16
N_NODES = 50000
N_EDGES = 800000
F_IN, F, N_CLASSES = 128, 64, 40
NCORES = 8
BLKS = 392                     # global dst blocks (50176 rows)
NROWS = BLKS * 128
SHARD_BLKS = BLKS // NCORES    # 49
SHARD = SHARD_BLKS * 128       # 6272
TPC = 32                       # tiles per gather chunk
SEGS = [23, 20, 6]             # split-RS segments (blocks per core, sum 49)
SEGJ = [0, 23, 43, 49]         # cumulative block boundaries within a shard
SEGP = [0, 184, 344, 392]      # cumulative position boundaries (x NCORES)
PSB = 8                        # dst blocks per PSUM slab
TGRP = 24                      # table-write staging group size (mult of EB=4)
BAD = 999.0
DVE_OF_10 = 10                 # S-builds: this many of 10 on DVE, rest Pool

_cache = {}


def _balance_blocks(src, dst):
    """Per-shard node->block assignment balancing cnt[c][b] across src
    shards c, so the shared op schedule (padded to max_c) wastes fewer
    tiles. Returns permn: node row -> new row (stays within its shard)."""
    core_e = src // SHARD
    # per-node in-degree split by src shard: v[n, c]
    v = np.zeros((NROWS, NCORES), np.int64)
    np.add.at(v, (dst, core_e), 1)
    permn = np.zeros(NROWS, np.int64)
    for s in range(NCORES):
        lo = s * SHARD
        nodes = np.arange(lo, lo + SHARD)
        vs = v[nodes]                        # [SHARD, 8]
        order = np.argsort(-vs.sum(1), kind="stable")
        S = np.zeros((SHARD_BLKS, NCORES), np.int64)
        space = np.full(SHARD_BLKS, 128, np.int64)
        bin_of = np.zeros(SHARD, np.int64)
        curmax = np.zeros(SHARD_BLKS, np.int64)
        for n in order:
            cand = S + vs[n][None, :]
            newmax = cand.max(1)
            delta = newmax - curmax
            delta[space == 0] = 1 << 40
            bpick = int(np.argmin(delta + newmax * 1e-6))
            bin_of[n] = bpick
            S[bpick] += vs[n]
            curmax[bpick] = newmax[bpick]
            space[bpick] -= 1
        # rows within each bin in order of assignment
        fill = np.zeros(SHARD_BLKS, np.int64)
        for n in range(SHARD):
            b = bin_of[n]
            permn[lo + n] = lo + b * 128 + fill[b]
            fill[b] += 1
    return permn


def _prep(features, src, dst, W0, b0, W1, b1, W2, b2):
    src = src.astype(np.int64)
    dst = dst.astype(np.int64)

    deg = np.bincount(dst, minlength=NROWS).astype(np.float32)
    invd_n = (1.0 / np.maximum(deg, 1.0)).astype(np.float32)
    iv_edge = invd_n[dst]

    permn = _balance_blocks(src, dst)
    src = permn[src]
    dst = permn[dst]
    core = src // SHARD
    # position permutation: global block (c, j) -> partial-tensor position.
    # Each RS segment g holds every core's blocks j in [SEGJ[g], SEGJ[g+1])
    # contiguously (core-major), so the RS piece for core c is contiguous.
    gb = np.arange(BLKS)
    gc, gj = gb // SHARD_BLKS, gb % SHARD_BLKS
    pos_of_blk = np.zeros(BLKS, np.int64)
    segs_n = SEGS
    for g in range(len(segs_n)):
        m = (gj >= SEGJ[g]) & (gj < SEGJ[g + 1])
        pos_of_blk[m] = (SEGP[g] + gc[m] * segs_n[g] + (gj[m] - SEGJ[g]))
    posv = pos_of_blk[dst >> 7]
    order = np.lexsort((posv, core))
    src_s, dst_s, core_s = src[order], dst[order], core[order]
    blk = pos_of_blk[dst_s >> 7]
    slot = (dst_s & 127).astype(np.float32)
    iv_e = iv_edge[order]
    loc = src_s - core_s * SHARD

    cnt = np.zeros((NCORES, BLKS), np.int64)
    np.add.at(cnt, (core_s, blk), 1)
    n_b = cnt.max(axis=0)
    S_off = np.concatenate([[0], np.cumsum(n_b)[:-1]])
    NS = int(n_b.sum())
    T = (NS + 127) // 128
    NSP = T * 128

    idx_flat = np.zeros((NCORES, NSP), np.int16)
    sl_flat = np.full((NCORES, NSP), BAD, np.float32)
    iv_flat = np.zeros((NCORES, NSP), np.float32)
    grp = core_s * BLKS + blk
    grp_cnt = cnt.reshape(-1)
    grp_starts = np.concatenate([[0], np.cumsum(grp_cnt)[:-1]])
    rank = np.arange(len(src_s)) - grp_starts[grp]
    pos = S_off[blk] + rank
    idx_flat[core_s, pos] = loc.astype(np.int16)
    sl_flat[core_s, pos] = slot
    iv_flat[core_s, pos] = iv_e

    # shared op schedule: one S-matmul per (tile, block) overlap
    ops = []
    blk_first = np.zeros(BLKS, np.int64)
    blk_nops = np.zeros(BLKS, np.int64)
    for b in range(BLKS):
        if n_b[b] == 0:
            continue
        t0 = int(S_off[b]) // 128
        t1 = int(S_off[b] + n_b[b] - 1) // 128
        blk_first[b] = len(ops)
        blk_nops[b] = t1 - t0 + 1
        for t in range(t0, t1 + 1):
            ops.append((t, b))
    NOPS = len(ops)

    sl_cols = np.full((NCORES, 128, NOPS), BAD, np.float32)
    iv_cols = np.zeros((NCORES, 128, NOPS), np.float32)
    for o, (t, b) in enumerate(ops):
        s0 = t * 128
        lo = max(int(S_off[b]), s0)
        hi = min(int(S_off[b] + n_b[b]), s0 + 128)
        sl_cols[:, lo - s0:hi - s0, o] = sl_flat[:, lo:hi]
        iv_cols[:, lo - s0:hi - s0, o] = iv_flat[:, lo:hi]

    idxd = np.stack([np.tile(idx_flat[c].reshape(-1, 16).T, (8, 1))
                     for c in range(NCORES)])          # [NCORES, 128, T*8]

    # column-permute features so table row permn[n] holds node n
    xTp = np.zeros((F_IN, NCORES * SHARD), dtype=BF)
    xTp[:, permn[:N_NODES]] = np.ascontiguousarray(features.T).astype(BF)

    W2p = np.zeros((F, F), np.float32)
    W2p[:, :N_CLASSES] = np.asarray(W2, np.float32)[:, :N_CLASSES]
    b2v = np.asarray(b2, np.float32).reshape(-1)
    b2p = np.zeros((F,), np.float32)
    b2p[:min(len(b2v), F)] = b2v[:min(len(b2v), F)]

    in_maps = []
    for c in range(NCORES):
        in_maps.append({
            "xT": np.ascontiguousarray(xTp[:, c * SHARD:(c + 1) * SHARD]),
            "idx": np.ascontiguousarray(idxd[c]),
            "sl": np.ascontiguousarray(sl_cols[c]),
            "iv": np.ascontiguousarray(iv_cols[c]),
            "W0b": np.asarray(W0, np.float32).astype(BF),
            "W1b": np.asarray(W1, np.float32).astype(BF),
            "W2b": W2p.astype(BF),
            "b0": np.asarray(b0, np.float32).reshape(F, 1),
            "b1": np.asarray(b1, np.float32).reshape(F, 1),
            "b2bc": np.tile(b2p, (128, 1)),
            "iota": np.tile(np.arange(128, dtype=np.float32),
                            (128, 1)).astype(BF),
            "ident": np.eye(128, dtype=np.float32).astype(BF),
        })
    sched = {"T": T, "NOPS": NOPS, "ops": ops,
             "blk_first": blk_first, "blk_nops": blk_nops, "permn": permn}
    return in_maps, sched


def _build(sched):
    T, NOPS = sched["T"], sched["NOPS"]
    ops = sched["ops"]
    blk_first, blk_nops = sched["blk_first"], sched["blk_nops"]

    nc = bacc.Bacc("TRN2", num_devices=NCORES,
                   dynamic_dma_scratch_size=65536)
    dt = mybir.dt
    f32, bf16, i16 = dt.float32, dt.float16, dt.int16

    xT_in = nc.dram_tensor("xT", [F_IN, SHARD], bf16, kind="ExternalInput")
    idx_in = nc.dram_tensor("idx", [128, T * 8], i16, kind="ExternalInput")
    sl_in = nc.dram_tensor("sl", [128, NOPS], f32, kind="ExternalInput")
    iv_in = nc.dram_tensor("iv", [128, NOPS], f32, kind="ExternalInput")
    W0_in = nc.dram_tensor("W0b", [F_IN, F], bf16, kind="ExternalInput")
    W1_in = nc.dram_tensor("W1b", [F, F], bf16, kind="ExternalInput")
    W2_in = nc.dram_tensor("W2b", [F, F], bf16, kind="ExternalInput")
    b0_in = nc.dram_tensor("b0", [F, 1], f32, kind="ExternalInput")
    b1_in = nc.dram_tensor("b1", [F, 1], f32, kind="ExternalInput")
    b2_in = nc.dram_tensor("b2bc", [128, F], f32, kind="ExternalInput")
    iota_in = nc.dram_tensor("iota", [128, 128], bf16, kind="ExternalInput")
    ident_in = nc.dram_tensor("ident", [128, 128], bf16, kind="ExternalInput")
    out = nc.dram_tensor("out", [SHARD, N_CLASSES], bf16, kind="ExternalOutput")

    n_chunks = (T + TPC - 1) // TPC

    with tile.TileContext(nc) as tc:
        with tc.tile_pool(name="const", bufs=1) as cp, \
             tc.tile_pool(name="dram", bufs=1, space="DRAM") as dram, \
             tc.tile_pool(name="msg", bufs=4) as mp, \
             tc.tile_pool(name="stl", bufs=40) as spl, \
             tc.tile_pool(name="pstg", bufs=3) as pstg, \
             tc.tile_pool(name="tstg", bufs=2) as tstg, \
             tc.tile_pool(name="epi", bufs=4) as ep, \
             tc.tile_pool(name="aggp", bufs=2, space="PSUM") as pp, \
             tc.tile_pool(name="trp", bufs=2, space="PSUM") as pt, \
             tc.tile_pool(name="prp", bufs=2, space="PSUM") as pp2:

            xT_sb = cp.tile([F_IN, SHARD], bf16)
            nc.sync.dma_start(xT_sb[:], xT_in[:])
            W0_t = cp.tile([F_IN, F], bf16)
            nc.sync.dma_start(W0_t[:], W0_in[:])
            iota_t = cp.tile([128, 128], bf16)
            nc.sync.dma_start(iota_t[:], iota_in[:])

            # tables: [SHARD, 128] fp16 rows (256B gather granules); only
            # cols 0:64 are ever written / read by compute.
            tbls = [dram.tile([SHARD, 128], bf16, tag=f"tbl{l}",
                              name=f"tbl{l}") for l in range(3)]
            FL = [F, F, N_CLASSES]         # live feature width per layer
            # partial slabs: per segment g, [NCORES, 128, n_g*Fl]; RS piece
            # for core c is exactly slab [c].
            parts = [[dram.tile([NCORES * 128, SEGS[g] * FL[l]], bf16,
                                tag=f"part{l}_{g}", name=f"part{l}_{g}")
                      for g in range(len(SEGS))] for l in range(3)]
            agg_d = [[dram.tile([128, SEGS[g] * FL[l]], bf16,
                                tag=f"agg{l}_{g}", name=f"agg{l}_{g}")
                      for g in range(len(SEGS))] for l in range(3)]

            # ---- Phase A: tbl0 = X @ W0 (local shard) ----
            stg = None
            for j in range(SHARD_BLKS):
                yp = pp2.tile([128, 4 * F], f32, tag="prj4")
                nc.tensor.matmul(yp[:, 0:F], xT_sb[:, j * 128:(j + 1) * 128],
                                 W0_t[:], start=True, stop=True)
                gi = j % TGRP
                if gi == 0:
                    stg = tstg.tile([128, TGRP, F], bf16, tag="tstg")
                if j % 2 == 0:
                    nc.scalar.activation(stg[:, gi, :], yp[:, 0:F],
                                         mybir.ActivationFunctionType.Identity,
                                         bias=0.0, scale=1.0)
                else:
                    nc.vector.tensor_copy(stg[:, gi, :], yp[:, 0:F])
                if gi == TGRP - 1 or j == SHARD_BLKS - 1:
                    g0 = j - gi
                    dst_ap = tbls[0][g0 * 128:(j + 1) * 128, 0:F] \
                        .rearrange("(g p) c -> p g c", p=128)
                    nc.sync.dma_start(dst_ap, stg[:, 0:gi + 1, :])

            ident_t = cp.tile([128, 128], bf16)
            nc.sync.dma_start(ident_t[:], ident_in[:])
            W1_t = cp.tile([F, F], bf16)
            nc.sync.dma_start(W1_t[:], W1_in[:])
            W2_t = cp.tile([F, F], bf16)
            nc.sync.dma_start(W2_t[:], W2_in[:])
            b0_t = cp.tile([F, 1], f32)
            nc.sync.dma_start(b0_t[:], b0_in[:])
            b1_t = cp.tile([F, 1], f32)
            nc.sync.dma_start(b1_t[:], b1_in[:])
            b2_t = cp.tile([128, F], f32)
            nc.sync.dma_start(b2_t[:], b2_in[:])
            idx_sb = cp.tile([128, T * 8], i16)
            nc.sync.dma_start(idx_sb[:], idx_in[:])
            sl_sb = cp.tile([128, NOPS], f32)
            nc.sync.dma_start(sl_sb[:], sl_in[:])
            iv_sb = cp.tile([128, NOPS], f32)
            nc.sync.dma_start(iv_sb[:], iv_in[:])

            # ---- Layers ----
            for l in range(3):
                tbl = tbls[l]
                # gather chunks are issued lazily as the op walk reaches them
                msgs = {}

                def ensure_chunk(ch):
                    if ch in msgs:
                        return
                    nt = min(TPC, T - ch * TPC)
                    msg = mp.tile([128, nt, 128], bf16, tag="msg")
                    nc.gpsimd.dma_gather(
                        msg[:], tbl[:],
                        idx_sb[:, ch * TPC * 8: ch * TPC * 8 + nt * 8],
                        num_idxs=nt * 128, num_idxs_reg=nt * 128,
                        elem_size=128, single_packet=False)
                    msgs[ch] = msg

                Fl = FL[l]

                def emit_epilogue(g):
                    # ---- per-segment epilogue on my shard; emitted half a
                    # segment AFTER its RS was issued so in-order engines
                    # don't stall on the RS. DMAs ride the ACT HWDGE ring
                    # so the SP ring stays walk-only. ----
                    n_g = SEGS[g]
                    agg_sb = ep.tile([128, n_g, Fl], bf16, tag="aggsb")
                    nc.scalar.dma_start(
                        agg_sb[:].rearrange("p g c -> p (g c)"),
                        agg_d[l][g][:])
                    if l < 2:
                        W_next = W1_t if l == 0 else W2_t
                        b_cur = b0_t if l == 0 else b1_t
                        stg2 = ep.tile([128, n_g, F], bf16, tag="tstg2")
                        EB = 4
                        for i0 in range(0, n_g, EB):
                            w = min(EB, n_g - i0)
                            hp = pt.tile([F, EB * 128], bf16, tag="hT")
                            for u in range(w):
                                nc.tensor.transpose(
                                    hp[:, u * 128:(u + 1) * 128],
                                    agg_sb[:, i0 + u, :], ident_t[:])
                            hT = ep.tile([F, EB * 128], bf16, tag="hTs")
                            nc.scalar.activation(
                                hT[:, 0:w * 128], hp[:, 0:w * 128],
                                mybir.ActivationFunctionType.Relu,
                                bias=b_cur[:, 0:1], scale=1.0)
                            tp = pp2.tile([128, EB * F], f32, tag="prj4")
                            for u in range(w):
                                nc.tensor.matmul(
                                    tp[:, u * F:(u + 1) * F],
                                    hT[:, u * 128:(u + 1) * 128],
                                    W_next[:], start=True, stop=True)
                            if (i0 // EB) % 2 == 0:
                                nc.vector.tensor_copy(
                                    stg2[:, i0:i0 + w, :].rearrange(
                                        "p g c -> p (g c)"),
                                    tp[:, 0:w * F])
                            else:
                                nc.scalar.activation(
                                    stg2[:, i0:i0 + w, :].rearrange(
                                        "p g c -> p (g c)"),
                                    tp[:, 0:w * F],
                                    mybir.ActivationFunctionType.Identity,
                                    bias=0.0, scale=1.0)
                        lo, hi = SEGJ[g], SEGJ[g + 1]
                        dst_ap = tbls[l + 1][lo * 128:hi * 128, 0:F] \
                            .rearrange("(g p) c -> p g c", p=128)
                        nc.scalar.dma_start(dst_ap, stg2[:])
                    else:
                        # bias add + store output for this segment
                        ostg = ep.tile([128, n_g, N_CLASSES], bf16,
                                       tag="ostg")
                        for i in range(n_g):
                            eng = nc.vector if i % 2 == 0 else nc.gpsimd
                            eng.tensor_tensor(
                                ostg[:, i, :], agg_sb[:, i, :],
                                b2_t[:, 0:N_CLASSES],
                                mybir.AluOpType.add)
                        lo, hi = SEGJ[g], SEGJ[g + 1]
                        dst_ap = out[lo * 128:hi * 128, :] \
                            .rearrange("(g p) c -> p g c", p=128)
                        nc.scalar.dma_start(dst_ap, ostg[:])

                for g in range(len(SEGS)):
                    n_g = SEGS[g]
                    for c in range(NCORES):
                        if g == 1 and c == 4:
                            emit_epilogue(0)
                        stg_t = pstg.tile([128, n_g, Fl], bf16, tag="pstg")
                        jj = 0
                        while jj < n_g:
                            w = min(PSB, n_g - jj)
                            ps = pp.tile([128, PSB * Fl], f32, tag="agg")
                            empties = []
                            for u in range(w):
                                b = SEGP[g] + c * n_g + jj + u
                                sl_ps = ps[:, u * Fl:(u + 1) * Fl]
                                if blk_nops[b] == 0:
                                    empties.append(u)
                                    continue
                                o0 = int(blk_first[b])
                                nops = int(blk_nops[b])
                                for k in range(nops):
                                    o = o0 + k
                                    t, _b = ops[o]
                                    ch = t // TPC
                                    ensure_chunk(ch)
                                    # keep 2 chunks of gather in flight ahead
                                    for la in (1, 2):
                                        if ch + la < n_chunks:
                                            ensure_chunk(ch + la)
                                    S = spl.tile([128, 128], bf16, tag="S")
                                    eng = nc.vector if (o % 10) < DVE_OF_10 \
                                        else nc.gpsimd
                                    eng.tensor_scalar(
                                        S[:], iota_t[:], sl_sb[:, o:o + 1],
                                        iv_sb[:, o:o + 1],
                                        mybir.AluOpType.is_equal,
                                        mybir.AluOpType.mult)
                                    nc.tensor.matmul(
                                        sl_ps, S[:],
                                        msgs[ch][:, t % TPC, 0:Fl],
                                        start=(k == 0), stop=(k == nops - 1))
                            # evacuate the whole slab in one ACT op
                            if len(empties) < w:
                                nc.scalar.activation(
                                    stg_t[:, jj:jj + w, :].rearrange(
                                        "p g c -> p (g c)"),
                                    ps[:, 0:w * Fl],
                                    mybir.ActivationFunctionType.Identity,
                                    bias=0.0, scale=1.0)
                            for u in empties:
                                nc.vector.memset(stg_t[:, jj + u, :], 0.0)
                            jj += w
                        nc.sync.dma_start(
                            parts[l][g][c * 128:(c + 1) * 128, :],
                            stg_t[:].rearrange("p g c -> p (g c)"))
                    nc.gpsimd.collective_compute(
                        "ReduceScatter", mybir.AluOpType.add,
                        replica_groups=[list(range(NCORES))],
                        ins=[parts[l][g][:].opt()],
                        outs=[agg_d[l][g][:].opt()])
                for g in range(1, len(SEGS)):
                    emit_epilogue(g)

    nc.compile()
    return nc


def kernel(features, src, dst, W0, b0, W1, b1, W2, b2):
    features = np.asarray(features, dtype=np.float32)
    src = np.asarray(src).astype(np.int64)
    dst = np.asarray(dst).astype(np.int64)
    in_maps, sched = _prep(features, src, dst,
                           np.asarray(W0), np.asarray(b0), np.asarray(W1),
                           np.asarray(b1), np.asarray(W2), np.asarray(b2))
    key = (sched["T"], sched["NOPS"],
           hash(sched["blk_first"].tobytes()),
           hash(sched["blk_nops"].tobytes()))
    if _cache.get("key") != key:
        _cache["nc"] = _build(sched)
        _cache["key"] = key
    nc = _cache["nc"]
    res = run_bass_kernel_spmd(nc, in_maps, core_ids=list(range(NCORES)))
    full = np.concatenate([np.asarray(res.results[c]["out"]).astype(np.float32)
                           for c in range(NCORES)], axis=0)
    # rows are in permuted (balanced-block) order; node n sits at permn[n]
    return np.ascontiguousarray(full[sched["permn"][:N_NODES], :N_CLASSES])


# revision 28
# speedup vs baseline: 1.0141x; 1.0141x over previous
"""GCN (3-layer GraphConv, norm='right') — 8-core SPMD Trainium2 Bass kernel.

Strategy (src-sharded edges + per-layer split ReduceScatter):
  Nodes are split into 8 contiguous shards of 49 blocks (6272 rows). Core c
  owns edges whose SRC lies in its shard, so every gather reads only the
  core-local projected table (single int16 index stream, 256B fp16 rows).
  Per layer, each core aggregates its edges' messages into a FULL 392-block
  partial table (one-hot S-matmuls with inv_deg folded in, fp16), then a
  3-segment ReduceScatter(add) — early segments issued mid-layer — sums
  partials and hands each core its dst shard. The epilogue applies
  bias/relu and the next layer's projection locally; no AllGather is ever
  needed because next-layer gathers only read local rows.

Perf notes (cost-model driven):
  - dma_gather descriptors are 256B (hard API floor) -> gather dominates DMA.
  - partial tables live in per-(segment) slab tensors [8, 128, n_g*64] so the
    staged write is one fat-descriptor DMA per (segment, core) and the RS
    piece for core c is exactly slab c; agg comes back partition-major.
  - PSUM accumulates 8 dst blocks per [128, 512] bank; one ACT activation
    evacuates all 8 (amortizes the ~185ns ACT init).
  - tables store real data in cols 0:64 of 128-wide rows; only those 128B
    are ever written (gathers read the full 256B row, upper half junk).
  - S-builds: ~1/10 on Pool (gpsimd), rest on DVE; Pool also runs gather
    desc-gen (994ns fixed per call -> TPC=48 tiles per gather).
"""
import numpy as np
import ml_dtypes

import concourse.bass as bass
import concourse.tile as tile
from concourse import bacc, mybir
from concourse.bass_utils import run_bass_kernel_spmd

BF = np.float16
N_NODES = 50000
N_EDGES = 800000
F_IN, F, N_CLASSES = 128, 64, 40
NCORES = 8
BLKS = 392                     # global dst blocks (50176 rows)
NROWS = BLKS * 128
SHARD_BLKS = BLKS // NCORES    # 49
SHARD = SHARD_BLKS * 128       # 6272
TPC = 24                       # tiles per gather chunk
SEGS = [23, 20, 6]             # split-RS segments (blocks per core, sum 49)
SEGJ = [0, 23, 43, 49]         # cumulative block boundaries within a shard
SEGP = [0, 184, 344, 392]      # cumulative position boundaries (x NCORES)
PSB = 8                        # dst blocks per PSUM slab
TGRP = 24                      # table-write staging group size (mult of EB=4)
BAD = 999.0
DVE_OF_10 = 10                 # S-builds: this many of 10 on DVE, rest Pool
SPL_BUFS = 40                  # S-tile pool depth
AGGP_BUFS = 2                  # PSUM slab pool depth
LEAD = (8, 8)                  # small leading gather chunks per layer

_cache = {}


def _balance_blocks(src, dst):
    """Per-shard node->block assignment balancing cnt[c][b] across src
    shards c, so the shared op schedule (padded to max_c) wastes fewer
    tiles. Returns permn: node row -> new row (stays within its shard)."""
    core_e = src // SHARD
    # per-node in-degree split by src shard: v[n, c]
    v = np.zeros((NROWS, NCORES), np.int64)
    np.add.at(v, (dst, core_e), 1)
    permn = np.zeros(NROWS, np.int64)
    for s in range(NCORES):
        lo = s * SHARD
        nodes = np.arange(lo, lo + SHARD)
        vs = v[nodes]                        # [SHARD, 8]
        order = np.argsort(-vs.sum(1), kind="stable")
        S = np.zeros((SHARD_BLKS, NCORES), np.int64)
        space = np.full(SHARD_BLKS, 128, np.int64)
        bin_of = np.zeros(SHARD, np.int64)
        curmax = np.zeros(SHARD_BLKS, np.int64)
        for n in order:
            cand = S + vs[n][None, :]
            newmax = cand.max(1)
            delta = newmax - curmax
            delta[space == 0] = 1 << 40
            bpick = int(np.argmin(delta + newmax * 1e-6))
            bin_of[n] = bpick
            S[bpick] += vs[n]
            curmax[bpick] = newmax[bpick]
            space[bpick] -= 1
        # rows within each bin in order of assignment
        fill = np.zeros(SHARD_BLKS, np.int64)
        for n in range(SHARD):
            b = bin_of[n]
            permn[lo + n] = lo + b * 128 + fill[b]
            fill[b] += 1
    return permn


def _prep(features, src, dst, W0, b0, W1, b1, W2, b2):
    src = src.astype(np.int64)
    dst = dst.astype(np.int64)

    deg = np.bincount(dst, minlength=NROWS).astype(np.float32)
    invd_n = (1.0 / np.maximum(deg, 1.0)).astype(np.float32)
    iv_edge = invd_n[dst]

    permn = _balance_blocks(src, dst)
    src = permn[src]
    dst = permn[dst]
    core = src // SHARD
    # position permutation: global block (c, j) -> partial-tensor position.
    # Each RS segment g holds every core's blocks j in [SEGJ[g], SEGJ[g+1])
    # contiguously (core-major), so the RS piece for core c is contiguous.
    gb = np.arange(BLKS)
    gc, gj = gb // SHARD_BLKS, gb % SHARD_BLKS
    pos_of_blk = np.zeros(BLKS, np.int64)
    segs_n = SEGS
    for g in range(len(segs_n)):
        m = (gj >= SEGJ[g]) & (gj < SEGJ[g + 1])
        pos_of_blk[m] = (SEGP[g] + gc[m] * segs_n[g] + (gj[m] - SEGJ[g]))
    posv = pos_of_blk[dst >> 7]
    order = np.lexsort((posv, core))
    src_s, dst_s, core_s = src[order], dst[order], core[order]
    blk = pos_of_blk[dst_s >> 7]
    slot = (dst_s & 127).astype(np.float32)
    iv_e = iv_edge[order]
    loc = src_s - core_s * SHARD

    cnt = np.zeros((NCORES, BLKS), np.int64)
    np.add.at(cnt, (core_s, blk), 1)
    n_b = cnt.max(axis=0)
    S_off = np.concatenate([[0], np.cumsum(n_b)[:-1]])
    NS = int(n_b.sum())
    T = (NS + 127) // 128
    NSP = T * 128

    idx_flat = np.zeros((NCORES, NSP), np.int16)
    sl_flat = np.full((NCORES, NSP), BAD, np.float32)
    iv_flat = np.zeros((NCORES, NSP), np.float32)
    grp = core_s * BLKS + blk
    grp_cnt = cnt.reshape(-1)
    grp_starts = np.concatenate([[0], np.cumsum(grp_cnt)[:-1]])
    rank = np.arange(len(src_s)) - grp_starts[grp]
    pos = S_off[blk] + rank
    idx_flat[core_s, pos] = loc.astype(np.int16)
    sl_flat[core_s, pos] = slot
    iv_flat[core_s, pos] = iv_e

    # shared op schedule: one S-matmul per (tile, block) overlap
    ops = []
    blk_first = np.zeros(BLKS, np.int64)
    blk_nops = np.zeros(BLKS, np.int64)
    for b in range(BLKS):
        if n_b[b] == 0:
            continue
        t0 = int(S_off[b]) // 128
        t1 = int(S_off[b] + n_b[b] - 1) // 128
        blk_first[b] = len(ops)
        blk_nops[b] = t1 - t0 + 1
        for t in range(t0, t1 + 1):
            ops.append((t, b))
    NOPS = len(ops)

    sl_cols = np.full((NCORES, 128, NOPS), BAD, np.float32)
    iv_cols = np.zeros((NCORES, 128, NOPS), np.float32)
    for o, (t, b) in enumerate(ops):
        s0 = t * 128
        lo = max(int(S_off[b]), s0)
        hi = min(int(S_off[b] + n_b[b]), s0 + 128)
        sl_cols[:, lo - s0:hi - s0, o] = sl_flat[:, lo:hi]
        iv_cols[:, lo - s0:hi - s0, o] = iv_flat[:, lo:hi]

    idxd = np.stack([np.tile(idx_flat[c].reshape(-1, 16).T, (8, 1))
                     for c in range(NCORES)])          # [NCORES, 128, T*8]

    # column-permute features so table row permn[n] holds node n
    xTp = np.zeros((F_IN, NCORES * SHARD), dtype=BF)
    xTp[:, permn[:N_NODES]] = np.ascontiguousarray(features.T).astype(BF)

    W2p = np.zeros((F, F), np.float32)
    W2p[:, :N_CLASSES] = np.asarray(W2, np.float32)[:, :N_CLASSES]
    b2v = np.asarray(b2, np.float32).reshape(-1)
    b2p = np.zeros((F,), np.float32)
    b2p[:min(len(b2v), F)] = b2v[:min(len(b2v), F)]

    in_maps = []
    for c in range(NCORES):
        in_maps.append({
            "xT": np.ascontiguousarray(xTp[:, c * SHARD:(c + 1) * SHARD]),
            "idx": np.ascontiguousarray(idxd[c]),
            "sl": np.ascontiguousarray(sl_cols[c]),
            "iv": np.ascontiguousarray(iv_cols[c]),
            "W0b": np.asarray(W0, np.float32).astype(BF),
            "W1b": np.asarray(W1, np.float32).astype(BF),
            "W2b": W2p.astype(BF),
            "b0": np.asarray(b0, np.float32).reshape(F, 1),
            "b1": np.asarray(b1, np.float32).reshape(F, 1),
            "b2bc": np.tile(b2p, (128, 1)),
            "iota": np.tile(np.arange(128, dtype=np.float32),
                            (128, 1)).astype(BF),
            "ident": np.eye(128, dtype=np.float32).astype(BF),
        })
    sched = {"T": T, "NOPS": NOPS, "ops": ops,
             "blk_first": blk_first, "blk_nops": blk_nops, "permn": permn}
    return in_maps, sched


def _build(sched):
    T, NOPS = sched["T"], sched["NOPS"]
    ops = sched["ops"]
    blk_first, blk_nops = sched["blk_first"], sched["blk_nops"]

    nc = bacc.Bacc("TRN2", num_devices=NCORES,
                   dynamic_dma_scratch_size=65536)
    dt = mybir.dt
    f32, bf16, i16 = dt.float32, dt.float16, dt.int16

    xT_in = nc.dram_tensor("xT", [F_IN, SHARD], bf16, kind="ExternalInput")
    idx_in = nc.dram_tensor("idx", [128, T * 8], i16, kind="ExternalInput")
    sl_in = nc.dram_tensor("sl", [128, NOPS], f32, kind="ExternalInput")
    iv_in = nc.dram_tensor("iv", [128, NOPS], f32, kind="ExternalInput")
    W0_in = nc.dram_tensor("W0b", [F_IN, F], bf16, kind="ExternalInput")
    W1_in = nc.dram_tensor("W1b", [F, F], bf16, kind="ExternalInput")
    W2_in = nc.dram_tensor("W2b", [F, F], bf16, kind="ExternalInput")
    b0_in = nc.dram_tensor("b0", [F, 1], f32, kind="ExternalInput")
    b1_in = nc.dram_tensor("b1", [F, 1], f32, kind="ExternalInput")
    b2_in = nc.dram_tensor("b2bc", [128, F], f32, kind="ExternalInput")
    iota_in = nc.dram_tensor("iota", [128, 128], bf16, kind="ExternalInput")
    ident_in = nc.dram_tensor("ident", [128, 128], bf16, kind="ExternalInput")
    out = nc.dram_tensor("out", [SHARD, N_CLASSES], bf16, kind="ExternalOutput")


    with tile.TileContext(nc) as tc:
        with tc.tile_pool(name="const", bufs=1) as cp, \
             tc.tile_pool(name="dram", bufs=1, space="DRAM") as dram, \
             tc.tile_pool(name="msg", bufs=4) as mp, \
             tc.tile_pool(name="stl", bufs=SPL_BUFS) as spl, \
             tc.tile_pool(name="pstg", bufs=3) as pstg, \
             tc.tile_pool(name="tstg", bufs=2) as tstg, \
             tc.tile_pool(name="epi", bufs=4) as ep, \
             tc.tile_pool(name="aggp", bufs=AGGP_BUFS, space="PSUM") as pp, \
             tc.tile_pool(name="trp", bufs=2, space="PSUM") as pt, \
             tc.tile_pool(name="prp", bufs=2, space="PSUM") as pp2:

            xT_sb = cp.tile([F_IN, SHARD], bf16)
            nc.sync.dma_start(xT_sb[:], xT_in[:])
            W0_t = cp.tile([F_IN, F], bf16)
            nc.sync.dma_start(W0_t[:], W0_in[:])
            iota_t = cp.tile([128, 128], bf16)
            nc.sync.dma_start(iota_t[:], iota_in[:])

            # tables: [SHARD, 128] fp16 rows (256B gather granules); only
            # cols 0:64 are ever written / read by compute.
            tbls = [dram.tile([SHARD, 128], bf16, tag=f"tbl{l}",
                              name=f"tbl{l}") for l in range(3)]
            FL = [F, F, N_CLASSES]         # live feature width per layer
            # partial slabs: per segment g, [NCORES, 128, n_g*Fl]; RS piece
            # for core c is exactly slab [c].
            parts = [[dram.tile([NCORES * 128, SEGS[g] * FL[l]], bf16,
                                tag=f"part{l}_{g}", name=f"part{l}_{g}")
                      for g in range(len(SEGS))] for l in range(3)]
            agg_d = [[dram.tile([128, SEGS[g] * FL[l]], bf16,
                                tag=f"agg{l}_{g}", name=f"agg{l}_{g}")
                      for g in range(len(SEGS))] for l in range(3)]

            # ---- Phase A: tbl0 = X @ W0 (local shard) ----
            stg = None
            for j in range(SHARD_BLKS):
                yp = pp2.tile([128, 4 * F], f32, tag="prj4")
                nc.tensor.matmul(yp[:, 0:F], xT_sb[:, j * 128:(j + 1) * 128],
                                 W0_t[:], start=True, stop=True)
                gi = j % TGRP
                if gi == 0:
                    stg = tstg.tile([128, TGRP, F], bf16, tag="tstg")
                if j % 2 == 0:
                    nc.scalar.activation(stg[:, gi, :], yp[:, 0:F],
                                         mybir.ActivationFunctionType.Identity,
                                         bias=0.0, scale=1.0)
                else:
                    nc.vector.tensor_copy(stg[:, gi, :], yp[:, 0:F])
                if gi == TGRP - 1 or j == SHARD_BLKS - 1:
                    g0 = j - gi
                    dst_ap = tbls[0][g0 * 128:(j + 1) * 128, 0:F] \
                        .rearrange("(g p) c -> p g c", p=128)
                    nc.sync.dma_start(dst_ap, stg[:, 0:gi + 1, :])

            ident_t = cp.tile([128, 128], bf16)
            nc.sync.dma_start(ident_t[:], ident_in[:])
            W1_t = cp.tile([F, F], bf16)
            nc.sync.dma_start(W1_t[:], W1_in[:])
            W2_t = cp.tile([F, F], bf16)
            nc.sync.dma_start(W2_t[:], W2_in[:])
            b0_t = cp.tile([F, 1], f32)
            nc.sync.dma_start(b0_t[:], b0_in[:])
            b1_t = cp.tile([F, 1], f32)
            nc.sync.dma_start(b1_t[:], b1_in[:])
            b2_t = cp.tile([128, F], f32)
            nc.sync.dma_start(b2_t[:], b2_in[:])
            idx_sb = cp.tile([128, T * 8], i16)
            nc.sync.dma_start(idx_sb[:], idx_in[:])
            sl_sb = cp.tile([128, NOPS], f32)
            nc.sync.dma_start(sl_sb[:], sl_in[:])
            iv_sb = cp.tile([128, NOPS], f32)
            nc.sync.dma_start(iv_sb[:], iv_in[:])

            # variable gather chunking: small leading chunks so each layer's
            # gather pipeline ramps fast behind serial Pool desc-gen
            chunk_starts = [0]
            for sz in LEAD:
                if chunk_starts[-1] + sz < T:
                    chunk_starts.append(chunk_starts[-1] + sz)
            while chunk_starts[-1] + TPC < T:
                chunk_starts.append(chunk_starts[-1] + TPC)
            chunk_ends = chunk_starts[1:] + [T]
            chunk_of_tile = np.zeros(T, np.int64)
            for ci, (a, bnd) in enumerate(zip(chunk_starts, chunk_ends)):
                chunk_of_tile[a:bnd] = ci
            n_chunks = len(chunk_starts)

            # ---- Layers ----
            for l in range(3):
                tbl = tbls[l]
                # gather chunks are issued lazily as the op walk reaches them
                msgs = {}

                def ensure_chunk(ch):
                    if ch in msgs:
                        return
                    t0c, t1c = chunk_starts[ch], chunk_ends[ch]
                    nt = t1c - t0c
                    msg = mp.tile([128, TPC, 128], bf16, tag="msg")
                    nc.gpsimd.dma_gather(
                        msg[:, 0:nt, :], tbl[:],
                        idx_sb[:, t0c * 8: t0c * 8 + nt * 8],
                        num_idxs=nt * 128, num_idxs_reg=nt * 128,
                        elem_size=128, single_packet=False)
                    msgs[ch] = (msg, t0c)

                Fl = FL[l]

                def emit_epilogue(g):
                    # ---- per-segment epilogue on my shard; emitted half a
                    # segment AFTER its RS was issued so in-order engines
                    # don't stall on the RS. DMAs ride the ACT HWDGE ring
                    # so the SP ring stays walk-only. ----
                    n_g = SEGS[g]
                    agg_sb = ep.tile([128, n_g, Fl], bf16, tag="aggsb")
                    nc.scalar.dma_start(
                        agg_sb[:].rearrange("p g c -> p (g c)"),
                        agg_d[l][g][:])
                    if l < 2:
                        W_next = W1_t if l == 0 else W2_t
                        b_cur = b0_t if l == 0 else b1_t
                        stg2 = ep.tile([128, n_g, F], bf16, tag="tstg2")
                        EB = 4
                        for i0 in range(0, n_g, EB):
                            w = min(EB, n_g - i0)
                            hp = pt.tile([F, EB * 128], bf16, tag="hT")
                            for u in range(w):
                                nc.tensor.transpose(
                                    hp[:, u * 128:(u + 1) * 128],
                                    agg_sb[:, i0 + u, :], ident_t[:])
                            hT = ep.tile([F, EB * 128], bf16, tag="hTs")
                            nc.scalar.activation(
                                hT[:, 0:w * 128], hp[:, 0:w * 128],
                                mybir.ActivationFunctionType.Relu,
                                bias=b_cur[:, 0:1], scale=1.0)
                            tp = pp2.tile([128, EB * F], f32, tag="prj4")
                            for u in range(w):
                                nc.tensor.matmul(
                                    tp[:, u * F:(u + 1) * F],
                                    hT[:, u * 128:(u + 1) * 128],
                                    W_next[:], start=True, stop=True)
                            if (i0 // EB) % 2 == 0:
                                nc.vector.tensor_copy(
                                    stg2[:, i0:i0 + w, :].rearrange(
                                        "p g c -> p (g c)"),
                                    tp[:, 0:w * F])
                            else:
                                nc.scalar.activation(
                                    stg2[:, i0:i0 + w, :].rearrange(
                                        "p g c -> p (g c)"),
                                    tp[:, 0:w * F],
                                    mybir.ActivationFunctionType.Identity,
                                    bias=0.0, scale=1.0)
                        lo, hi = SEGJ[g], SEGJ[g + 1]
                        dst_ap = tbls[l + 1][lo * 128:hi * 128, 0:F] \
                            .rearrange("(g p) c -> p g c", p=128)
                        nc.scalar.dma_start(dst_ap, stg2[:])
                    else:
                        # bias add + store output for this segment
                        ostg = ep.tile([128, n_g, N_CLASSES], bf16,
                                       tag="ostg")
                        for i in range(n_g):
                            eng = nc.vector if i % 2 == 0 else nc.gpsimd
                            eng.tensor_tensor(
                                ostg[:, i, :], agg_sb[:, i, :],
                                b2_t[:, 0:N_CLASSES],
                                mybir.AluOpType.add)
                        lo, hi = SEGJ[g], SEGJ[g + 1]
                        dst_ap = out[lo * 128:hi * 128, :] \
                            .rearrange("(g p) c -> p g c", p=128)
                        nc.scalar.dma_start(dst_ap, ostg[:])

                for g in range(len(SEGS)):
                    n_g = SEGS[g]
                    for c in range(NCORES):
                        if g == 1 and c == 4:
                            emit_epilogue(0)
                        stg_t = pstg.tile([128, n_g, Fl], bf16, tag="pstg")
                        jj = 0
                        while jj < n_g:
                            w = min(PSB, n_g - jj)
                            ps = pp.tile([128, PSB * Fl], f32, tag="agg")
                            empties = []
                            for u in range(w):
                                b = SEGP[g] + c * n_g + jj + u
                                sl_ps = ps[:, u * Fl:(u + 1) * Fl]
                                if blk_nops[b] == 0:
                                    empties.append(u)
                                    continue
                                o0 = int(blk_first[b])
                                nops = int(blk_nops[b])
                                for k in range(nops):
                                    o = o0 + k
                                    t, _b = ops[o]
                                    ch = int(chunk_of_tile[t])
                                    ensure_chunk(ch)
                                    # keep 2 chunks of gather in flight ahead
                                    for la in (1, 2):
                                        if ch + la < n_chunks:
                                            ensure_chunk(ch + la)
                                    S = spl.tile([128, 128], bf16, tag="S")
                                    eng = nc.vector if (o % 10) < DVE_OF_10 \
                                        else nc.gpsimd
                                    eng.tensor_scalar(
                                        S[:], iota_t[:], sl_sb[:, o:o + 1],
                                        iv_sb[:, o:o + 1],
                                        mybir.AluOpType.is_equal,
                                        mybir.AluOpType.mult)
                                    nc.tensor.matmul(
                                        sl_ps, S[:],
                                        msgs[ch][0][:, t - msgs[ch][1],
                                                     0:Fl],
                                        start=(k == 0), stop=(k == nops - 1))
                            # evacuate the whole slab in one ACT op
                            if len(empties) < w:
                                nc.scalar.activation(
                                    stg_t[:, jj:jj + w, :].rearrange(
                                        "p g c -> p (g c)"),
                                    ps[:, 0:w * Fl],
                                    mybir.ActivationFunctionType.Identity,
                                    bias=0.0, scale=1.0)
                            for u in empties:
                                nc.vector.memset(stg_t[:, jj + u, :], 0.0)
                            jj += w
                        nc.sync.dma_start(
                            parts[l][g][c * 128:(c + 1) * 128, :],
                            stg_t[:].rearrange("p g c -> p (g c)"))
                    nc.gpsimd.collective_compute(
                        "ReduceScatter", mybir.AluOpType.add,
                        replica_groups=[list(range(NCORES))],
                        ins=[parts[l][g][:].opt()],
                        outs=[agg_d[l][g][:].opt()])
                for g in range(1, len(SEGS)):
                    emit_epilogue(g)

    nc.compile()
    return nc


def kernel(features, src, dst, W0, b0, W1, b1, W2, b2):
    features = np.asarray(features, dtype=np.float32)
    src = np.asarray(src).astype(np.int64)
    dst = np.asarray(dst).astype(np.int64)
    in_maps, sched = _prep(features, src, dst,
                           np.asarray(W0), np.asarray(b0), np.asarray(W1),
                           np.asarray(b1), np.asarray(W2), np.asarray(b2))
    key = (sched["T"], sched["NOPS"],
           hash(sched["blk_first"].tobytes()),
           hash(sched["blk_nops"].tobytes()))
    if _cache.get("key") != key:
        _cache["nc"] = _build(sched)
        _cache["key"] = key
    nc = _cache["nc"]
    res = run_bass_kernel_spmd(nc, in_maps, core_ids=list(range(NCORES)))
    full = np.concatenate([np.asarray(res.results[c]["out"]).astype(np.float32)
                           for c in range(NCORES)], axis=0)
    # rows are in permuted (balanced-block) order; node n sits at permn[n]
    return np.ascontiguousarray(full[sched["permn"][:N_NODES], :N_CLASSES])


# revision 34
# speedup vs baseline: 1.0278x; 1.0135x over previous
"""GCN (3-layer GraphConv, norm='right') — 8-core SPMD Trainium2 Bass kernel.

Strategy (src-sharded edges + per-layer split ReduceScatter):
  Nodes are split into 8 contiguous shards of 49 blocks (6272 rows). Core c
  owns edges whose SRC lies in its shard, so every gather reads only the
  core-local projected table (single int16 index stream, 256B fp16 rows).
  Per layer, each core aggregates its edges' messages into a FULL 392-block
  partial table (one-hot S-matmuls with inv_deg folded in, fp16), then a
  3-segment ReduceScatter(add) — early segments issued mid-layer — sums
  partials and hands each core its dst shard. The epilogue applies
  bias/relu and the next layer's projection locally; no AllGather is ever
  needed because next-layer gathers only read local rows.

Perf notes (cost-model driven):
  - dma_gather descriptors are 256B (hard API floor) -> gather dominates DMA.
  - partial tables live in per-(segment) slab tensors [8, 128, n_g*64] so the
    staged write is one fat-descriptor DMA per (segment, core) and the RS
    piece for core c is exactly slab c; agg comes back partition-major.
  - PSUM accumulates 8 dst blocks per [128, 512] bank; one ACT activation
    evacuates all 8 (amortizes the ~185ns ACT init).
  - tables store real data in cols 0:64 of 128-wide rows; only those 128B
    are ever written (gathers read the full 256B row, upper half junk).
  - S-builds: ~1/10 on Pool (gpsimd), rest on DVE; Pool also runs gather
    desc-gen (994ns fixed per call -> TPC=48 tiles per gather).
"""
import numpy as np
import ml_dtypes

import concourse.bass as bass
import concourse.tile as tile
from concourse import bacc, mybir
from concourse.bass_utils import run_bass_kernel_spmd

BF = np.float16
N_NODES = 50000
N_EDGES = 800000
F_IN, F, N_CLASSES = 128, 64, 40
NCORES = 8
BLKS = 392                     # global dst blocks (50176 rows)
NROWS = BLKS * 128
SHARD_BLKS = BLKS // NCORES    # 49
SHARD = SHARD_BLKS * 128       # 6272
TPC = 24                       # tiles per gather chunk
SEGS = [23, 20, 6]             # split-RS segments (blocks per core, sum 49)
SEGJ = [0, 23, 43, 49]         # cumulative block boundaries within a shard
SEGP = [0, 184, 344, 392]      # cumulative position boundaries (x NCORES)
PSB = 8                        # dst blocks per PSUM slab
TGRP = 24                      # table-write staging group size (mult of EB=4)
BAD = 999.0
DVE_OF_10 = 10                 # S-builds: this many of 10 on DVE, rest Pool
SPL_BUFS = 40                  # S-tile pool depth
AGGP_BUFS = 2                  # PSUM slab pool depth
LEAD = (8, 8)                  # small leading gather chunks per layer
EPI_C = 4                      # walk core index at which to emit prior-seg epilogue

_cache = {}


EARLY_SHARDS = 1               # walk positions of shards 0..E-1's seg0 bins
                               # are "early"; seg2-row nodes must not point
                               # into them so lead gathers skip seg2 rows


def _balance_blocks(src, dst):
    """Per-shard node->block assignment balancing cnt[c][b] across src
    shards c, so the shared op schedule (padded to max_c) wastes fewer
    tiles. Returns permn: node row -> new row (stays within its shard).

    Constraint: nodes placed in seg2 bins (j >= SEGJ[-2]) must have no
    out-edge whose dst lands in an "early" bin (seg0 bins of shards
    0..EARLY_SHARDS-1). Early walk tiles then never gather seg2 rows, so
    the next layer's lead gathers only depend on seg0+seg1 table writes."""
    core_e = src // SHARD
    # per-node in-degree split by src shard: v[n, c]
    v = np.zeros((NROWS, NCORES), np.int64)
    np.add.at(v, (dst, core_e), 1)

    seg2_lo = SEGJ[-2]          # first seg2 bin index within a shard
    n_seg2 = SEGS[-1]

    def assign(vs, forbid_seg2, locked=None):
        """Greedy LPT over bins; nodes with forbid_seg2 can't go to seg2
        bins. `locked`: optional pre-assignment (bin id or -1) — locked
        nodes keep their bin, consuming space/sums first."""
        S = np.zeros((SHARD_BLKS, NCORES), np.int64)
        space = np.full(SHARD_BLKS, 128, np.int64)
        bin_of = np.full(len(vs), -1, np.int64)
        if locked is not None:
            for n in np.where(locked >= 0)[0]:
                b = locked[n]
                bin_of[n] = b
                S[b] += vs[n]
                space[b] -= 1
        curmax = S.max(1)
        order = np.argsort(-vs.sum(1), kind="stable")
        big = 1 << 40
        qual_left = int((~forbid_seg2[bin_of < 0]).sum())
        for n in order:
            if bin_of[n] >= 0:
                continue
            cand = S + vs[n][None, :]
            newmax = cand.max(1)
            delta = (newmax - curmax).astype(np.float64)
            delta[space == 0] = big
            if forbid_seg2[n]:
                delta[seg2_lo:] = big
            else:
                # feasibility: if the remaining qualifying nodes are only
                # just enough to fill seg2 bins, force them there
                if qual_left <= int(space[seg2_lo:].sum()):
                    delta2 = delta.copy()
                    delta2[:seg2_lo] = big
                    if delta2.min() < big:
                        delta = delta2
                qual_left -= 1
            bpick = int(np.argmin(delta + newmax * 1e-6))
            bin_of[n] = bpick
            S[bpick] += vs[n]
            curmax[bpick] = newmax[bpick]
            space[bpick] -= 1
        return bin_of

    permn = np.zeros(NROWS, np.int64)
    bin_of_shard = [None] * NCORES
    nofb = np.zeros(SHARD, bool)

    # pass 1: unconstrained assignment for shards 0..E-1 defines the
    # frozen "early" node set (their seg0 bins).
    for s in range(EARLY_SHARDS):
        lo = s * SHARD
        bin_of_shard[s] = assign(v[lo:lo + SHARD], nofb)
    early = np.zeros(NROWS, bool)
    for s in range(EARLY_SHARDS):
        lo = s * SHARD
        early[lo + np.where(bin_of_shard[s] < SEGJ[1])[0]] = True

    def forbid_for(s):
        # node n (row offset) forbidden from seg2 if any out-edge is early
        lo = s * SHARD
        bad = np.zeros(SHARD, bool)
        m = (src >= lo) & (src < lo + SHARD)
        e_src = src[m] - lo
        e_dst_early = early[dst[m]]
        np.logical_or.at(bad, e_src[e_dst_early], True)
        return bad

    # pass 2: shards 0..E-1 keep their seg0 bins locked and redistribute
    # the rest under the seg2 constraint; other shards assign fresh.
    for s in range(NCORES):
        lo = s * SHARD
        vs = v[lo:lo + SHARD]
        forbid = forbid_for(s)
        if s < EARLY_SHARDS:
            locked = np.where(bin_of_shard[s] < SEGJ[1],
                              bin_of_shard[s], -1)
            free_qual = int((~forbid[locked < 0]).sum())
            if free_qual < n_seg2 * 128 + 32:
                forbid[:] = False   # infeasible -> give up the guarantee
            bin_of_shard[s] = assign(vs, forbid, locked=locked)
        else:
            if (~forbid).sum() < n_seg2 * 128 + 32:
                forbid[:] = False   # infeasible -> give up the guarantee
            bin_of_shard[s] = assign(vs, forbid)

    for s in range(NCORES):
        lo = s * SHARD
        bin_of = bin_of_shard[s]
        fill = np.zeros(SHARD_BLKS, np.int64)
        for n in range(SHARD):
            b = bin_of[n]
            permn[lo + n] = lo + b * 128 + fill[b]
            fill[b] += 1
    return permn


def _prep(features, src, dst, W0, b0, W1, b1, W2, b2):
    src = src.astype(np.int64)
    dst = dst.astype(np.int64)

    deg = np.bincount(dst, minlength=NROWS).astype(np.float32)
    invd_n = (1.0 / np.maximum(deg, 1.0)).astype(np.float32)
    iv_edge = invd_n[dst]

    permn = _balance_blocks(src, dst)
    src = permn[src]
    dst = permn[dst]
    core = src // SHARD
    # position permutation: global block (c, j) -> partial-tensor position.
    # Each RS segment g holds every core's blocks j in [SEGJ[g], SEGJ[g+1])
    # contiguously (core-major), so the RS piece for core c is contiguous.
    gb = np.arange(BLKS)
    gc, gj = gb // SHARD_BLKS, gb % SHARD_BLKS
    pos_of_blk = np.zeros(BLKS, np.int64)
    segs_n = SEGS
    for g in range(len(segs_n)):
        m = (gj >= SEGJ[g]) & (gj < SEGJ[g + 1])
        pos_of_blk[m] = (SEGP[g] + gc[m] * segs_n[g] + (gj[m] - SEGJ[g]))
    posv = pos_of_blk[dst >> 7]
    order = np.lexsort((posv, core))
    src_s, dst_s, core_s = src[order], dst[order], core[order]
    blk = pos_of_blk[dst_s >> 7]
    slot = (dst_s & 127).astype(np.float32)
    iv_e = iv_edge[order]
    loc = src_s - core_s * SHARD

    cnt = np.zeros((NCORES, BLKS), np.int64)
    np.add.at(cnt, (core_s, blk), 1)
    n_b = cnt.max(axis=0)
    S_off = np.concatenate([[0], np.cumsum(n_b)[:-1]])
    NS = int(n_b.sum())
    T = (NS + 127) // 128
    NSP = T * 128

    idx_flat = np.zeros((NCORES, NSP), np.int16)
    sl_flat = np.full((NCORES, NSP), BAD, np.float32)
    iv_flat = np.zeros((NCORES, NSP), np.float32)
    grp = core_s * BLKS + blk
    grp_cnt = cnt.reshape(-1)
    grp_starts = np.concatenate([[0], np.cumsum(grp_cnt)[:-1]])
    rank = np.arange(len(src_s)) - grp_starts[grp]
    pos = S_off[blk] + rank
    idx_flat[core_s, pos] = loc.astype(np.int16)
    sl_flat[core_s, pos] = slot
    iv_flat[core_s, pos] = iv_e

    # shared op schedule: one S-matmul per (tile, block) overlap
    ops = []
    blk_first = np.zeros(BLKS, np.int64)
    blk_nops = np.zeros(BLKS, np.int64)
    for b in range(BLKS):
        if n_b[b] == 0:
            continue
        t0 = int(S_off[b]) // 128
        t1 = int(S_off[b] + n_b[b] - 1) // 128
        blk_first[b] = len(ops)
        blk_nops[b] = t1 - t0 + 1
        for t in range(t0, t1 + 1):
            ops.append((t, b))
    NOPS = len(ops)

    sl_cols = np.full((NCORES, 128, NOPS), BAD, np.float32)
    iv_cols = np.zeros((NCORES, 128, NOPS), np.float32)
    for o, (t, b) in enumerate(ops):
        s0 = t * 128
        lo = max(int(S_off[b]), s0)
        hi = min(int(S_off[b] + n_b[b]), s0 + 128)
        sl_cols[:, lo - s0:hi - s0, o] = sl_flat[:, lo:hi]
        iv_cols[:, lo - s0:hi - s0, o] = iv_flat[:, lo:hi]

    idxd = np.stack([np.tile(idx_flat[c].reshape(-1, 16).T, (8, 1))
                     for c in range(NCORES)])          # [NCORES, 128, T*8]

    # column-permute features so table row permn[n] holds node n
    xTp = np.zeros((F_IN, NCORES * SHARD), dtype=BF)
    xTp[:, permn[:N_NODES]] = np.ascontiguousarray(features.T).astype(BF)

    W2p = np.zeros((F, F), np.float32)
    W2p[:, :N_CLASSES] = np.asarray(W2, np.float32)[:, :N_CLASSES]
    b2v = np.asarray(b2, np.float32).reshape(-1)
    b2p = np.zeros((F,), np.float32)
    b2p[:min(len(b2v), F)] = b2v[:min(len(b2v), F)]

    in_maps = []
    for c in range(NCORES):
        in_maps.append({
            "xT": np.ascontiguousarray(xTp[:, c * SHARD:(c + 1) * SHARD]),
            "idx": np.ascontiguousarray(idxd[c]),
            "sl": np.ascontiguousarray(sl_cols[c]),
            "iv": np.ascontiguousarray(iv_cols[c]),
            "W0b": np.asarray(W0, np.float32).astype(BF),
            "W1b": np.asarray(W1, np.float32).astype(BF),
            "W2b": W2p.astype(BF),
            "b0": np.asarray(b0, np.float32).reshape(F, 1),
            "b1": np.asarray(b1, np.float32).reshape(F, 1),
            "b2bc": np.tile(b2p, (128, 1)),
            "iota": np.tile(np.arange(128, dtype=np.float32),
                            (128, 1)).astype(BF),
            "ident": np.eye(128, dtype=np.float32).astype(BF),
        })
    # tiles below early_tiles only touch blocks in the "early" positions
    # (seg0 of shards 0..EARLY_SHARDS-1) whose edges provably avoid seg2
    # table rows (rows >= SEGJ[-2]*128) -> their gathers only depend on
    # the seg0+seg1 table writes of the previous layer.
    P_early = EARLY_SHARDS * SEGS[0]
    early_tiles = int(S_off[P_early]) // 128 if P_early < BLKS else 0
    lead_rows = SEGJ[-2] * 128
    while early_tiles > 0 and \
            int(idx_flat[:, :early_tiles * 128].max()) >= lead_rows:
        early_tiles -= 1
    sched = {"T": T, "NOPS": NOPS, "ops": ops,
             "blk_first": blk_first, "blk_nops": blk_nops, "permn": permn,
             "early_tiles": early_tiles}
    return in_maps, sched


def _build(sched):
    T, NOPS = sched["T"], sched["NOPS"]
    early_tiles = sched.get("early_tiles", 0)
    lead_rows = SEGJ[-2] * 128
    ops = sched["ops"]
    blk_first, blk_nops = sched["blk_first"], sched["blk_nops"]

    nc = bacc.Bacc("TRN2", num_devices=NCORES,
                   dynamic_dma_scratch_size=65536)
    dt = mybir.dt
    f32, bf16, i16 = dt.float32, dt.float16, dt.int16

    xT_in = nc.dram_tensor("xT", [F_IN, SHARD], bf16, kind="ExternalInput")
    idx_in = nc.dram_tensor("idx", [128, T * 8], i16, kind="ExternalInput")
    sl_in = nc.dram_tensor("sl", [128, NOPS], f32, kind="ExternalInput")
    iv_in = nc.dram_tensor("iv", [128, NOPS], f32, kind="ExternalInput")
    W0_in = nc.dram_tensor("W0b", [F_IN, F], bf16, kind="ExternalInput")
    W1_in = nc.dram_tensor("W1b", [F, F], bf16, kind="ExternalInput")
    W2_in = nc.dram_tensor("W2b", [F, F], bf16, kind="ExternalInput")
    b0_in = nc.dram_tensor("b0", [F, 1], f32, kind="ExternalInput")
    b1_in = nc.dram_tensor("b1", [F, 1], f32, kind="ExternalInput")
    b2_in = nc.dram_tensor("b2bc", [128, F], f32, kind="ExternalInput")
    iota_in = nc.dram_tensor("iota", [128, 128], bf16, kind="ExternalInput")
    ident_in = nc.dram_tensor("ident", [128, 128], bf16, kind="ExternalInput")
    out = nc.dram_tensor("out", [SHARD, N_CLASSES], bf16, kind="ExternalOutput")


    with tile.TileContext(nc) as tc:
        with tc.tile_pool(name="const", bufs=1) as cp, \
             tc.tile_pool(name="dram", bufs=1, space="DRAM") as dram, \
             tc.tile_pool(name="msg", bufs=4) as mp, \
             tc.tile_pool(name="stl", bufs=SPL_BUFS) as spl, \
             tc.tile_pool(name="pstg", bufs=3) as pstg, \
             tc.tile_pool(name="tstg", bufs=2) as tstg, \
             tc.tile_pool(name="epi", bufs=4) as ep, \
             tc.tile_pool(name="aggp", bufs=AGGP_BUFS, space="PSUM") as pp, \
             tc.tile_pool(name="trp", bufs=2, space="PSUM") as pt, \
             tc.tile_pool(name="prp", bufs=2, space="PSUM") as pp2:

            xT_sb = cp.tile([F_IN, SHARD], bf16)
            nc.sync.dma_start(xT_sb[:], xT_in[:])
            W0_t = cp.tile([F_IN, F], bf16)
            nc.sync.dma_start(W0_t[:], W0_in[:])
            iota_t = cp.tile([128, 128], bf16)
            nc.sync.dma_start(iota_t[:], iota_in[:])

            # tables: [SHARD, 128] fp16 rows (256B gather granules); only
            # cols 0:64 are ever written / read by compute.
            tbls = [dram.tile([SHARD, 128], bf16, tag=f"tbl{l}",
                              name=f"tbl{l}") for l in range(3)]
            FL = [F, F, N_CLASSES]         # live feature width per layer
            # partial slabs: per segment g, [NCORES, 128, n_g*Fl]; RS piece
            # for core c is exactly slab [c].
            parts = [[dram.tile([NCORES * 128, SEGS[g] * FL[l]], bf16,
                                tag=f"part{l}_{g}", name=f"part{l}_{g}")
                      for g in range(len(SEGS))] for l in range(3)]
            agg_d = [[dram.tile([128, SEGS[g] * FL[l]], bf16,
                                tag=f"agg{l}_{g}", name=f"agg{l}_{g}")
                      for g in range(len(SEGS))] for l in range(3)]

            # ---- Phase A: tbl0 = X @ W0 (local shard) ----
            stg = None
            for j in range(SHARD_BLKS):
                yp = pp2.tile([128, 4 * F], f32, tag="prj4")
                nc.tensor.matmul(yp[:, 0:F], xT_sb[:, j * 128:(j + 1) * 128],
                                 W0_t[:], start=True, stop=True)
                gi = j % TGRP
                if gi == 0:
                    stg = tstg.tile([128, TGRP, F], bf16, tag="tstg")
                if j % 2 == 0:
                    nc.scalar.activation(stg[:, gi, :], yp[:, 0:F],
                                         mybir.ActivationFunctionType.Identity,
                                         bias=0.0, scale=1.0)
                else:
                    nc.vector.tensor_copy(stg[:, gi, :], yp[:, 0:F])
                if gi == TGRP - 1 or j == SHARD_BLKS - 1:
                    g0 = j - gi
                    dst_ap = tbls[0][g0 * 128:(j + 1) * 128, 0:F] \
                        .rearrange("(g p) c -> p g c", p=128)
                    nc.sync.dma_start(dst_ap, stg[:, 0:gi + 1, :])

            ident_t = cp.tile([128, 128], bf16)
            nc.sync.dma_start(ident_t[:], ident_in[:])
            W1_t = cp.tile([F, F], bf16)
            nc.sync.dma_start(W1_t[:], W1_in[:])
            W2_t = cp.tile([F, F], bf16)
            nc.sync.dma_start(W2_t[:], W2_in[:])
            b0_t = cp.tile([F, 1], f32)
            nc.sync.dma_start(b0_t[:], b0_in[:])
            b1_t = cp.tile([F, 1], f32)
            nc.sync.dma_start(b1_t[:], b1_in[:])
            b2_t = cp.tile([128, F], f32)
            nc.sync.dma_start(b2_t[:], b2_in[:])
            idx_sb = cp.tile([128, T * 8], i16)
            nc.sync.dma_start(idx_sb[:], idx_in[:])
            sl_sb = cp.tile([128, NOPS], f32)
            nc.sync.dma_start(sl_sb[:], sl_in[:])
            iv_sb = cp.tile([128, NOPS], f32)
            nc.sync.dma_start(iv_sb[:], iv_in[:])

            # variable gather chunking: small leading chunks so each layer's
            # gather pipeline ramps fast behind serial Pool desc-gen
            bset = {0, T}
            cur = 0
            for sz in LEAD:
                cur += sz
                if cur < T:
                    bset.add(cur)
            while cur + TPC < T:
                cur += TPC
                bset.add(cur)
            if 0 < early_tiles < T:
                bset.add(early_tiles)
            chunk_starts = sorted(bset - {T})
            chunk_ends = chunk_starts[1:] + [T]
            chunk_of_tile = np.zeros(T, np.int64)
            for ci, (a, bnd) in enumerate(zip(chunk_starts, chunk_ends)):
                chunk_of_tile[a:bnd] = ci
            n_chunks = len(chunk_starts)

            # ---- Layers ----
            for l in range(3):
                tbl = tbls[l]
                # gather chunks are issued lazily as the op walk reaches them
                msgs = {}

                def ensure_chunk(ch):
                    if ch in msgs:
                        return
                    t0c, t1c = chunk_starts[ch], chunk_ends[ch]
                    nt = t1c - t0c
                    msg = mp.tile([128, TPC, 128], bf16, tag="msg")
                    src_ap = tbl[0:lead_rows, :] if t1c <= early_tiles \
                        else tbl[:]
                    nc.gpsimd.dma_gather(
                        msg[:, 0:nt, :], src_ap,
                        idx_sb[:, t0c * 8: t0c * 8 + nt * 8],
                        num_idxs=nt * 128, num_idxs_reg=nt * 128,
                        elem_size=128, single_packet=False)
                    msgs[ch] = (msg, t0c)

                Fl = FL[l]

                def emit_epilogue(g):
                    # ---- per-segment epilogue on my shard; emitted half a
                    # segment AFTER its RS was issued so in-order engines
                    # don't stall on the RS. DMAs ride the ACT HWDGE ring
                    # so the SP ring stays walk-only. ----
                    n_g = SEGS[g]
                    agg_sb = ep.tile([128, n_g, Fl], bf16, tag="aggsb")
                    nc.scalar.dma_start(
                        agg_sb[:].rearrange("p g c -> p (g c)"),
                        agg_d[l][g][:])
                    if l < 2:
                        W_next = W1_t if l == 0 else W2_t
                        b_cur = b0_t if l == 0 else b1_t
                        stg2 = ep.tile([128, n_g, F], bf16, tag="tstg2")
                        EB = 4
                        for i0 in range(0, n_g, EB):
                            w = min(EB, n_g - i0)
                            hp = pt.tile([F, EB * 128], bf16, tag="hT")
                            for u in range(w):
                                nc.tensor.transpose(
                                    hp[:, u * 128:(u + 1) * 128],
                                    agg_sb[:, i0 + u, :], ident_t[:])
                            hT = ep.tile([F, EB * 128], bf16, tag="hTs")
                            nc.scalar.activation(
                                hT[:, 0:w * 128], hp[:, 0:w * 128],
                                mybir.ActivationFunctionType.Relu,
                                bias=b_cur[:, 0:1], scale=1.0)
                            tp = pp2.tile([128, EB * F], f32, tag="prj4")
                            for u in range(w):
                                nc.tensor.matmul(
                                    tp[:, u * F:(u + 1) * F],
                                    hT[:, u * 128:(u + 1) * 128],
                                    W_next[:], start=True, stop=True)
                            if (i0 // EB) % 2 == 0:
                                nc.vector.tensor_copy(
                                    stg2[:, i0:i0 + w, :].rearrange(
                                        "p g c -> p (g c)"),
                                    tp[:, 0:w * F])
                            else:
                                nc.scalar.activation(
                                    stg2[:, i0:i0 + w, :].rearrange(
                                        "p g c -> p (g c)"),
                                    tp[:, 0:w * F],
                                    mybir.ActivationFunctionType.Identity,
                                    bias=0.0, scale=1.0)
                        lo, hi = SEGJ[g], SEGJ[g + 1]
                        dst_ap = tbls[l + 1][lo * 128:hi * 128, 0:F] \
                            .rearrange("(g p) c -> p g c", p=128)
                        nc.scalar.dma_start(dst_ap, stg2[:])
                    else:
                        # bias add + store output for this segment
                        ostg = ep.tile([128, n_g, N_CLASSES], bf16,
                                       tag="ostg")
                        for i in range(n_g):
                            eng = nc.vector if i % 2 == 0 else nc.gpsimd
                            eng.tensor_tensor(
                                ostg[:, i, :], agg_sb[:, i, :],
                                b2_t[:, 0:N_CLASSES],
                                mybir.AluOpType.add)
                        lo, hi = SEGJ[g], SEGJ[g + 1]
                        dst_ap = out[lo * 128:hi * 128, :] \
                            .rearrange("(g p) c -> p g c", p=128)
                        nc.scalar.dma_start(dst_ap, ostg[:])

                emitted = set()
                for g in range(len(SEGS)):
                    n_g = SEGS[g]
                    for c in range(NCORES):
                        if g >= 1 and c == EPI_C and (g - 1) \
                                not in emitted and g - 1 < len(SEGS) - 2:
                            emitted.add(g - 1)
                            emit_epilogue(g - 1)
                        stg_t = pstg.tile([128, n_g, Fl], bf16, tag="pstg")
                        jj = 0
                        while jj < n_g:
                            w = min(PSB, n_g - jj)
                            ps = pp.tile([128, PSB * Fl], f32, tag="agg")
                            empties = []
                            for u in range(w):
                                b = SEGP[g] + c * n_g + jj + u
                                sl_ps = ps[:, u * Fl:(u + 1) * Fl]
                                if blk_nops[b] == 0:
                                    empties.append(u)
                                    continue
                                o0 = int(blk_first[b])
                                nops = int(blk_nops[b])
                                for k in range(nops):
                                    o = o0 + k
                                    t, _b = ops[o]
                                    ch = int(chunk_of_tile[t])
                                    ensure_chunk(ch)
                                    # keep 2 chunks of gather in flight ahead
                                    for la in (1, 2):
                                        if ch + la < n_chunks:
                                            ensure_chunk(ch + la)
                                    S = spl.tile([128, 128], bf16, tag="S")
                                    eng = nc.vector if (o % 10) < DVE_OF_10 \
                                        else nc.gpsimd
                                    eng.tensor_scalar(
                                        S[:], iota_t[:], sl_sb[:, o:o + 1],
                                        iv_sb[:, o:o + 1],
                                        mybir.AluOpType.is_equal,
                                        mybir.AluOpType.mult)
                                    nc.tensor.matmul(
                                        sl_ps, S[:],
                                        msgs[ch][0][:, t - msgs[ch][1],
                                                     0:Fl],
                                        start=(k == 0), stop=(k == nops - 1))
                            # evacuate the whole slab in one ACT op
                            if len(empties) < w:
                                nc.scalar.activation(
                                    stg_t[:, jj:jj + w, :].rearrange(
                                        "p g c -> p (g c)"),
                                    ps[:, 0:w * Fl],
                                    mybir.ActivationFunctionType.Identity,
                                    bias=0.0, scale=1.0)
                            for u in empties:
                                nc.vector.memset(stg_t[:, jj + u, :], 0.0)
                            jj += w
                        nc.sync.dma_start(
                            parts[l][g][c * 128:(c + 1) * 128, :],
                            stg_t[:].rearrange("p g c -> p (g c)"))
                    nc.gpsimd.collective_compute(
                        "ReduceScatter", mybir.AluOpType.add,
                        replica_groups=[list(range(NCORES))],
                        ins=[parts[l][g][:].opt()],
                        outs=[agg_d[l][g][:].opt()])
                for g in range(len(SEGS)):
                    if g not in emitted:
                        emit_epilogue(g)

    nc.compile()
    return nc


def kernel(features, src, dst, W0, b0, W1, b1, W2, b2):
    features = np.asarray(features, dtype=np.float32)
    src = np.asarray(src).astype(np.int64)
    dst = np.asarray(dst).astype(np.int64)
    in_maps, sched = _prep(features, src, dst,
                           np.asarray(W0), np.asarray(b0), np.asarray(W1),
                           np.asarray(b1), np.asarray(W2), np.asarray(b2))
    key = (sched["T"], sched["NOPS"],
           hash(sched["blk_first"].tobytes()),
           hash(sched["blk_nops"].tobytes()))
    if _cache.get("key") != key:
        _cache["nc"] = _build(sched)
        _cache["key"] = key
    nc = _cache["nc"]
    res = run_bass_kernel_spmd(nc, in_maps, core_ids=list(range(NCORES)))
    full = np.concatenate([np.asarray(res.results[c]["out"]).astype(np.float32)
                           for c in range(NCORES)], axis=0)
    # rows are in permuted (balanced-block) order; node n sits at permn[n]
    return np.ascontiguousarray(full[sched["permn"][:N_NODES], :N_CLASSES])


# revision 36
# speedup vs baseline: 1.0289x; 1.0011x over previous
"""GCN (3-layer GraphConv, norm='right') — 8-core SPMD Trainium2 Bass kernel.

Strategy (src-sharded edges + per-layer split ReduceScatter):
  Nodes are split into 8 contiguous shards of 49 blocks (6272 rows). Core c
  owns edges whose SRC lies in its shard, so every gather reads only the
  core-local projected table (single int16 index stream, 256B fp16 rows).
  Per layer, each core aggregates its edges' messages into a FULL 392-block
  partial table (one-hot S-matmuls with inv_deg folded in, fp16), then a
  3-segment ReduceScatter(add) — early segments issued mid-layer — sums
  partials and hands each core its dst shard. The epilogue applies
  bias/relu and the next layer's projection locally; no AllGather is ever
  needed because next-layer gathers only read local rows.

Perf notes (cost-model driven):
  - dma_gather descriptors are 256B (hard API floor) -> gather dominates DMA.
  - partial tables live in per-(segment) slab tensors [8, 128, n_g*64] so the
    staged write is one fat-descriptor DMA per (segment, core) and the RS
    piece for core c is exactly slab c; agg comes back partition-major.
  - PSUM accumulates 8 dst blocks per [128, 512] bank; one ACT activation
    evacuates all 8 (amortizes the ~185ns ACT init).
  - tables store real data in cols 0:64 of 128-wide rows; only those 128B
    are ever written (gathers read the full 256B row, upper half junk).
  - S-builds: ~1/10 on Pool (gpsimd), rest on DVE; Pool also runs gather
    desc-gen (994ns fixed per call -> TPC=48 tiles per gather).
"""
import numpy as np
import ml_dtypes

import concourse.bass as bass
import concourse.tile as tile
from concourse import bacc, mybir
from concourse.bass_utils import run_bass_kernel_spmd

BF = np.float16
N_NODES = 50000
N_EDGES = 800000
F_IN, F, N_CLASSES = 128, 64, 40
NCORES = 8
BLKS = 392                     # global dst blocks (50176 rows)
NROWS = BLKS * 128
SHARD_BLKS = BLKS // NCORES    # 49
SHARD = SHARD_BLKS * 128       # 6272
TPC = 24                       # tiles per gather chunk
SEGS = [22, 19, 8]             # split-RS segments (blocks per core, sum 49)
SEGJ = [0, 22, 41, 49]         # cumulative block boundaries within a shard
SEGP = [0, 176, 328, 392]      # cumulative position boundaries (x NCORES)
PSB = 8                        # dst blocks per PSUM slab
TGRP = 24                      # table-write staging group size (mult of EB=4)
BAD = 999.0
DVE_OF_10 = 10                 # S-builds: this many of 10 on DVE, rest Pool
SPL_BUFS = 40                  # S-tile pool depth
AGGP_BUFS = 2                  # PSUM slab pool depth
LEAD = (8, 8)                  # small leading gather chunks per layer
EPI_C = 4                      # walk core index at which to emit prior-seg epilogue

_cache = {}


EARLY_SHARDS = 1               # walk positions of shards 0..E-1's seg0 bins
                               # are "early"; seg2-row nodes must not point
                               # into them so lead gathers skip seg2 rows
EARLY_BINS = 22                # early bins per early shard (<= SEGS[0])


def _balance_blocks(src, dst):
    """Per-shard node->block assignment balancing cnt[c][b] across src
    shards c, so the shared op schedule (padded to max_c) wastes fewer
    tiles. Returns permn: node row -> new row (stays within its shard).

    Constraint: nodes placed in seg2 bins (j >= SEGJ[-2]) must have no
    out-edge whose dst lands in an "early" bin (seg0 bins of shards
    0..EARLY_SHARDS-1). Early walk tiles then never gather seg2 rows, so
    the next layer's lead gathers only depend on seg0+seg1 table writes."""
    core_e = src // SHARD
    # per-node in-degree split by src shard: v[n, c]
    v = np.zeros((NROWS, NCORES), np.int64)
    np.add.at(v, (dst, core_e), 1)

    seg2_lo = SEGJ[-2]          # first seg2 bin index within a shard
    n_seg2 = SEGS[-1]

    def assign(vs, forbid_seg2, locked=None):
        """Greedy LPT over bins; nodes with forbid_seg2 can't go to seg2
        bins. `locked`: optional pre-assignment (bin id or -1) — locked
        nodes keep their bin, consuming space/sums first."""
        S = np.zeros((SHARD_BLKS, NCORES), np.int64)
        space = np.full(SHARD_BLKS, 128, np.int64)
        bin_of = np.full(len(vs), -1, np.int64)
        if locked is not None:
            for n in np.where(locked >= 0)[0]:
                b = locked[n]
                bin_of[n] = b
                S[b] += vs[n]
                space[b] -= 1
        curmax = S.max(1)
        order = np.argsort(-vs.sum(1), kind="stable")
        big = 1 << 40
        qual_left = int((~forbid_seg2[bin_of < 0]).sum())
        for n in order:
            if bin_of[n] >= 0:
                continue
            cand = S + vs[n][None, :]
            newmax = cand.max(1)
            delta = (newmax - curmax).astype(np.float64)
            delta[space == 0] = big
            if forbid_seg2[n]:
                delta[seg2_lo:] = big
            else:
                # feasibility: if the remaining qualifying nodes are only
                # just enough to fill seg2 bins, force them there
                if qual_left <= int(space[seg2_lo:].sum()):
                    delta2 = delta.copy()
                    delta2[:seg2_lo] = big
                    if delta2.min() < big:
                        delta = delta2
                qual_left -= 1
            bpick = int(np.argmin(delta + newmax * 1e-6))
            bin_of[n] = bpick
            S[bpick] += vs[n]
            curmax[bpick] = newmax[bpick]
            space[bpick] -= 1
        return bin_of

    permn = np.zeros(NROWS, np.int64)
    bin_of_shard = [None] * NCORES
    nofb = np.zeros(SHARD, bool)

    # pass 1: unconstrained assignment for shards 0..E-1 defines the
    # frozen "early" node set (their seg0 bins).
    for s in range(EARLY_SHARDS):
        lo = s * SHARD
        bin_of_shard[s] = assign(v[lo:lo + SHARD], nofb)
    early = np.zeros(NROWS, bool)
    for s in range(EARLY_SHARDS):
        lo = s * SHARD
        early[lo + np.where(bin_of_shard[s] < EARLY_BINS)[0]] = True

    def forbid_for(s):
        # node n (row offset) forbidden from seg2 if any out-edge is early
        lo = s * SHARD
        bad = np.zeros(SHARD, bool)
        m = (src >= lo) & (src < lo + SHARD)
        e_src = src[m] - lo
        e_dst_early = early[dst[m]]
        np.logical_or.at(bad, e_src[e_dst_early], True)
        return bad

    # pass 2: shards 0..E-1 keep their seg0 bins locked and redistribute
    # the rest under the seg2 constraint; other shards assign fresh.
    for s in range(NCORES):
        lo = s * SHARD
        vs = v[lo:lo + SHARD]
        forbid = forbid_for(s)
        if s < EARLY_SHARDS:
            locked = np.where(bin_of_shard[s] < EARLY_BINS,
                              bin_of_shard[s], -1)
            free_qual = int((~forbid[locked < 0]).sum())
            if free_qual < n_seg2 * 128 + 32:
                forbid[:] = False   # infeasible -> give up the guarantee
            bin_of_shard[s] = assign(vs, forbid, locked=locked)
        else:
            if (~forbid).sum() < n_seg2 * 128 + 32:
                forbid[:] = False   # infeasible -> give up the guarantee
            bin_of_shard[s] = assign(vs, forbid)

    for s in range(NCORES):
        lo = s * SHARD
        bin_of = bin_of_shard[s]
        fill = np.zeros(SHARD_BLKS, np.int64)
        for n in range(SHARD):
            b = bin_of[n]
            permn[lo + n] = lo + b * 128 + fill[b]
            fill[b] += 1
    return permn


def _prep(features, src, dst, W0, b0, W1, b1, W2, b2):
    src = src.astype(np.int64)
    dst = dst.astype(np.int64)

    deg = np.bincount(dst, minlength=NROWS).astype(np.float32)
    invd_n = (1.0 / np.maximum(deg, 1.0)).astype(np.float32)
    iv_edge = invd_n[dst]

    permn = _balance_blocks(src, dst)
    src = permn[src]
    dst = permn[dst]
    core = src // SHARD
    # position permutation: global block (c, j) -> partial-tensor position.
    # Each RS segment g holds every core's blocks j in [SEGJ[g], SEGJ[g+1])
    # contiguously (core-major), so the RS piece for core c is contiguous.
    gb = np.arange(BLKS)
    gc, gj = gb // SHARD_BLKS, gb % SHARD_BLKS
    pos_of_blk = np.zeros(BLKS, np.int64)
    segs_n = SEGS
    for g in range(len(segs_n)):
        m = (gj >= SEGJ[g]) & (gj < SEGJ[g + 1])
        pos_of_blk[m] = (SEGP[g] + gc[m] * segs_n[g] + (gj[m] - SEGJ[g]))
    posv = pos_of_blk[dst >> 7]
    order = np.lexsort((posv, core))
    src_s, dst_s, core_s = src[order], dst[order], core[order]
    blk = pos_of_blk[dst_s >> 7]
    slot = (dst_s & 127).astype(np.float32)
    iv_e = iv_edge[order]
    loc = src_s - core_s * SHARD

    cnt = np.zeros((NCORES, BLKS), np.int64)
    np.add.at(cnt, (core_s, blk), 1)
    n_b = cnt.max(axis=0)
    S_off = np.concatenate([[0], np.cumsum(n_b)[:-1]])
    NS = int(n_b.sum())
    T = (NS + 127) // 128
    NSP = T * 128

    idx_flat = np.zeros((NCORES, NSP), np.int16)
    sl_flat = np.full((NCORES, NSP), BAD, np.float32)
    iv_flat = np.zeros((NCORES, NSP), np.float32)
    grp = core_s * BLKS + blk
    grp_cnt = cnt.reshape(-1)
    grp_starts = np.concatenate([[0], np.cumsum(grp_cnt)[:-1]])
    rank = np.arange(len(src_s)) - grp_starts[grp]
    pos = S_off[blk] + rank
    idx_flat[core_s, pos] = loc.astype(np.int16)
    sl_flat[core_s, pos] = slot
    iv_flat[core_s, pos] = iv_e

    # shared op schedule: one S-matmul per (tile, block) overlap
    ops = []
    blk_first = np.zeros(BLKS, np.int64)
    blk_nops = np.zeros(BLKS, np.int64)
    for b in range(BLKS):
        if n_b[b] == 0:
            continue
        t0 = int(S_off[b]) // 128
        t1 = int(S_off[b] + n_b[b] - 1) // 128
        blk_first[b] = len(ops)
        blk_nops[b] = t1 - t0 + 1
        for t in range(t0, t1 + 1):
            ops.append((t, b))
    NOPS = len(ops)

    sl_cols = np.full((NCORES, 128, NOPS), BAD, np.float32)
    iv_cols = np.zeros((NCORES, 128, NOPS), np.float32)
    for o, (t, b) in enumerate(ops):
        s0 = t * 128
        lo = max(int(S_off[b]), s0)
        hi = min(int(S_off[b] + n_b[b]), s0 + 128)
        sl_cols[:, lo - s0:hi - s0, o] = sl_flat[:, lo:hi]
        iv_cols[:, lo - s0:hi - s0, o] = iv_flat[:, lo:hi]

    idxd = np.stack([np.tile(idx_flat[c].reshape(-1, 16).T, (8, 1))
                     for c in range(NCORES)])          # [NCORES, 128, T*8]

    # column-permute features so table row permn[n] holds node n
    xTp = np.zeros((F_IN, NCORES * SHARD), dtype=BF)
    xTp[:, permn[:N_NODES]] = np.ascontiguousarray(features.T).astype(BF)

    W2p = np.zeros((F, F), np.float32)
    W2p[:, :N_CLASSES] = np.asarray(W2, np.float32)[:, :N_CLASSES]
    b2v = np.asarray(b2, np.float32).reshape(-1)
    b2p = np.zeros((F,), np.float32)
    b2p[:min(len(b2v), F)] = b2v[:min(len(b2v), F)]

    in_maps = []
    for c in range(NCORES):
        in_maps.append({
            "xT": np.ascontiguousarray(xTp[:, c * SHARD:(c + 1) * SHARD]),
            "idx": np.ascontiguousarray(idxd[c]),
            "sl": np.ascontiguousarray(sl_cols[c]),
            "iv": np.ascontiguousarray(iv_cols[c]),
            "W0b": np.asarray(W0, np.float32).astype(BF),
            "W1b": np.asarray(W1, np.float32).astype(BF),
            "W2b": W2p.astype(BF),
            "b0": np.asarray(b0, np.float32).reshape(F, 1),
            "b1": np.asarray(b1, np.float32).reshape(F, 1),
            "b2bc": np.tile(b2p, (128, 1)),
            "iota": np.tile(np.arange(128, dtype=np.float32),
                            (128, 1)).astype(BF),
            "ident": np.eye(128, dtype=np.float32).astype(BF),
        })
    # tiles below early_tiles only touch blocks in the "early" positions
    # (seg0 of shards 0..EARLY_SHARDS-1) whose edges provably avoid seg2
    # table rows (rows >= SEGJ[-2]*128) -> their gathers only depend on
    # the seg0+seg1 table writes of the previous layer.
    P_early = min(EARLY_BINS, SEGS[0]) * EARLY_SHARDS
    early_tiles = int(S_off[P_early]) // 128 if P_early < BLKS else 0
    lead_rows = SEGJ[-2] * 128
    while early_tiles > 0 and \
            int(idx_flat[:, :early_tiles * 128].max()) >= lead_rows:
        early_tiles -= 1
    sched = {"T": T, "NOPS": NOPS, "ops": ops,
             "blk_first": blk_first, "blk_nops": blk_nops, "permn": permn,
             "early_tiles": early_tiles}
    return in_maps, sched


def _build(sched):
    T, NOPS = sched["T"], sched["NOPS"]
    early_tiles = sched.get("early_tiles", 0)
    lead_rows = SEGJ[-2] * 128
    ops = sched["ops"]
    blk_first, blk_nops = sched["blk_first"], sched["blk_nops"]

    nc = bacc.Bacc("TRN2", num_devices=NCORES,
                   dynamic_dma_scratch_size=65536)
    dt = mybir.dt
    f32, bf16, i16 = dt.float32, dt.float16, dt.int16

    xT_in = nc.dram_tensor("xT", [F_IN, SHARD], bf16, kind="ExternalInput")
    idx_in = nc.dram_tensor("idx", [128, T * 8], i16, kind="ExternalInput")
    sl_in = nc.dram_tensor("sl", [128, NOPS], f32, kind="ExternalInput")
    iv_in = nc.dram_tensor("iv", [128, NOPS], f32, kind="ExternalInput")
    W0_in = nc.dram_tensor("W0b", [F_IN, F], bf16, kind="ExternalInput")
    W1_in = nc.dram_tensor("W1b", [F, F], bf16, kind="ExternalInput")
    W2_in = nc.dram_tensor("W2b", [F, F], bf16, kind="ExternalInput")
    b0_in = nc.dram_tensor("b0", [F, 1], f32, kind="ExternalInput")
    b1_in = nc.dram_tensor("b1", [F, 1], f32, kind="ExternalInput")
    b2_in = nc.dram_tensor("b2bc", [128, F], f32, kind="ExternalInput")
    iota_in = nc.dram_tensor("iota", [128, 128], bf16, kind="ExternalInput")
    ident_in = nc.dram_tensor("ident", [128, 128], bf16, kind="ExternalInput")
    out = nc.dram_tensor("out", [SHARD, N_CLASSES], bf16, kind="ExternalOutput")


    with tile.TileContext(nc) as tc:
        with tc.tile_pool(name="const", bufs=1) as cp, \
             tc.tile_pool(name="dram", bufs=1, space="DRAM") as dram, \
             tc.tile_pool(name="msg", bufs=4) as mp, \
             tc.tile_pool(name="stl", bufs=SPL_BUFS) as spl, \
             tc.tile_pool(name="pstg", bufs=3) as pstg, \
             tc.tile_pool(name="tstg", bufs=2) as tstg, \
             tc.tile_pool(name="epi", bufs=4) as ep, \
             tc.tile_pool(name="aggp", bufs=AGGP_BUFS, space="PSUM") as pp, \
             tc.tile_pool(name="trp", bufs=2, space="PSUM") as pt, \
             tc.tile_pool(name="prp", bufs=2, space="PSUM") as pp2:

            xT_sb = cp.tile([F_IN, SHARD], bf16)
            nc.sync.dma_start(xT_sb[:], xT_in[:])
            W0_t = cp.tile([F_IN, F], bf16)
            nc.sync.dma_start(W0_t[:], W0_in[:])
            iota_t = cp.tile([128, 128], bf16)
            nc.sync.dma_start(iota_t[:], iota_in[:])

            # tables: [SHARD, 128] fp16 rows (256B gather granules); only
            # cols 0:64 are ever written / read by compute.
            tbls = [dram.tile([SHARD, 128], bf16, tag=f"tbl{l}",
                              name=f"tbl{l}") for l in range(3)]
            FL = [F, F, N_CLASSES]         # live feature width per layer
            # partial slabs: per segment g, [NCORES, 128, n_g*Fl]; RS piece
            # for core c is exactly slab [c].
            parts = [[dram.tile([NCORES * 128, SEGS[g] * FL[l]], bf16,
                                tag=f"part{l}_{g}", name=f"part{l}_{g}")
                      for g in range(len(SEGS))] for l in range(3)]
            agg_d = [[dram.tile([128, SEGS[g] * FL[l]], bf16,
                                tag=f"agg{l}_{g}", name=f"agg{l}_{g}")
                      for g in range(len(SEGS))] for l in range(3)]

            # ---- Phase A: tbl0 = X @ W0 (local shard) ----
            stg = None
            for j in range(SHARD_BLKS):
                yp = pp2.tile([128, 4 * F], f32, tag="prj4")
                nc.tensor.matmul(yp[:, 0:F], xT_sb[:, j * 128:(j + 1) * 128],
                                 W0_t[:], start=True, stop=True)
                gi = j % TGRP
                if gi == 0:
                    stg = tstg.tile([128, TGRP, F], bf16, tag="tstg")
                if j % 2 == 0:
                    nc.scalar.activation(stg[:, gi, :], yp[:, 0:F],
                                         mybir.ActivationFunctionType.Identity,
                                         bias=0.0, scale=1.0)
                else:
                    nc.vector.tensor_copy(stg[:, gi, :], yp[:, 0:F])
                if gi == TGRP - 1 or j == SHARD_BLKS - 1:
                    g0 = j - gi
                    dst_ap = tbls[0][g0 * 128:(j + 1) * 128, 0:F] \
                        .rearrange("(g p) c -> p g c", p=128)
                    nc.sync.dma_start(dst_ap, stg[:, 0:gi + 1, :])

            ident_t = cp.tile([128, 128], bf16)
            nc.sync.dma_start(ident_t[:], ident_in[:])
            W1_t = cp.tile([F, F], bf16)
            nc.sync.dma_start(W1_t[:], W1_in[:])
            W2_t = cp.tile([F, F], bf16)
            nc.sync.dma_start(W2_t[:], W2_in[:])
            b0_t = cp.tile([F, 1], f32)
            nc.sync.dma_start(b0_t[:], b0_in[:])
            b1_t = cp.tile([F, 1], f32)
            nc.sync.dma_start(b1_t[:], b1_in[:])
            b2_t = cp.tile([128, F], f32)
            nc.sync.dma_start(b2_t[:], b2_in[:])
            idx_sb = cp.tile([128, T * 8], i16)
            nc.sync.dma_start(idx_sb[:], idx_in[:])
            sl_sb = cp.tile([128, NOPS], f32)
            nc.sync.dma_start(sl_sb[:], sl_in[:])
            iv_sb = cp.tile([128, NOPS], f32)
            nc.sync.dma_start(iv_sb[:], iv_in[:])

            # variable gather chunking: small leading chunks so each layer's
            # gather pipeline ramps fast behind serial Pool desc-gen
            bset = {0, T}
            cur = 0
            for sz in LEAD:
                cur += sz
                if cur < T:
                    bset.add(cur)
            while cur + TPC < T:
                cur += TPC
                bset.add(cur)
            if 0 < early_tiles < T:
                bset.add(early_tiles)
            chunk_starts = sorted(bset - {T})
            chunk_ends = chunk_starts[1:] + [T]
            chunk_of_tile = np.zeros(T, np.int64)
            for ci, (a, bnd) in enumerate(zip(chunk_starts, chunk_ends)):
                chunk_of_tile[a:bnd] = ci
            n_chunks = len(chunk_starts)

            # ---- Layers ----
            for l in range(3):
                tbl = tbls[l]
                # gather chunks are issued lazily as the op walk reaches them
                msgs = {}

                def ensure_chunk(ch):
                    if ch in msgs:
                        return
                    t0c, t1c = chunk_starts[ch], chunk_ends[ch]
                    nt = t1c - t0c
                    msg = mp.tile([128, TPC, 128], bf16, tag="msg")
                    src_ap = tbl[0:lead_rows, :] if t1c <= early_tiles \
                        else tbl[:]
                    nc.gpsimd.dma_gather(
                        msg[:, 0:nt, :], src_ap,
                        idx_sb[:, t0c * 8: t0c * 8 + nt * 8],
                        num_idxs=nt * 128, num_idxs_reg=nt * 128,
                        elem_size=128, single_packet=False)
                    msgs[ch] = (msg, t0c)

                Fl = FL[l]

                def emit_epilogue(g):
                    # ---- per-segment epilogue on my shard; emitted half a
                    # segment AFTER its RS was issued so in-order engines
                    # don't stall on the RS. DMAs ride the ACT HWDGE ring
                    # so the SP ring stays walk-only. ----
                    n_g = SEGS[g]
                    agg_sb = ep.tile([128, n_g, Fl], bf16, tag="aggsb")
                    nc.scalar.dma_start(
                        agg_sb[:].rearrange("p g c -> p (g c)"),
                        agg_d[l][g][:])
                    if l < 2:
                        W_next = W1_t if l == 0 else W2_t
                        b_cur = b0_t if l == 0 else b1_t
                        stg2 = ep.tile([128, n_g, F], bf16, tag="tstg2")
                        EB = 4
                        for i0 in range(0, n_g, EB):
                            w = min(EB, n_g - i0)
                            hp = pt.tile([F, EB * 128], bf16, tag="hT")
                            for u in range(w):
                                nc.tensor.transpose(
                                    hp[:, u * 128:(u + 1) * 128],
                                    agg_sb[:, i0 + u, :], ident_t[:])
                            hT = ep.tile([F, EB * 128], bf16, tag="hTs")
                            nc.scalar.activation(
                                hT[:, 0:w * 128], hp[:, 0:w * 128],
                                mybir.ActivationFunctionType.Relu,
                                bias=b_cur[:, 0:1], scale=1.0)
                            tp = pp2.tile([128, EB * F], f32, tag="prj4")
                            for u in range(w):
                                nc.tensor.matmul(
                                    tp[:, u * F:(u + 1) * F],
                                    hT[:, u * 128:(u + 1) * 128],
                                    W_next[:], start=True, stop=True)
                            if (i0 // EB) % 2 == 0:
                                nc.vector.tensor_copy(
                                    stg2[:, i0:i0 + w, :].rearrange(
                                        "p g c -> p (g c)"),
                                    tp[:, 0:w * F])
                            else:
                                nc.scalar.activation(
                                    stg2[:, i0:i0 + w, :].rearrange(
                                        "p g c -> p (g c)"),
                                    tp[:, 0:w * F],
                                    mybir.ActivationFunctionType.Identity,
                                    bias=0.0, scale=1.0)
                        lo, hi = SEGJ[g], SEGJ[g + 1]
                        dst_ap = tbls[l + 1][lo * 128:hi * 128, 0:F] \
                            .rearrange("(g p) c -> p g c", p=128)
                        nc.scalar.dma_start(dst_ap, stg2[:])
                    else:
                        # bias add + store output for this segment
                        ostg = ep.tile([128, n_g, N_CLASSES], bf16,
                                       tag="ostg")
                        for i in range(n_g):
                            eng = nc.vector if i % 2 == 0 else nc.gpsimd
                            eng.tensor_tensor(
                                ostg[:, i, :], agg_sb[:, i, :],
                                b2_t[:, 0:N_CLASSES],
                                mybir.AluOpType.add)
                        lo, hi = SEGJ[g], SEGJ[g + 1]
                        dst_ap = out[lo * 128:hi * 128, :] \
                            .rearrange("(g p) c -> p g c", p=128)
                        nc.scalar.dma_start(dst_ap, ostg[:])

                emitted = set()
                for g in range(len(SEGS)):
                    n_g = SEGS[g]
                    for c in range(NCORES):
                        if g >= 1 and c == EPI_C and (g - 1) \
                                not in emitted and g - 1 < len(SEGS) - 2:
                            emitted.add(g - 1)
                            emit_epilogue(g - 1)
                        stg_t = pstg.tile([128, n_g, Fl], bf16, tag="pstg")
                        jj = 0
                        while jj < n_g:
                            w = min(PSB, n_g - jj)
                            ps = pp.tile([128, PSB * Fl], f32, tag="agg")
                            empties = []
                            for u in range(w):
                                b = SEGP[g] + c * n_g + jj + u
                                sl_ps = ps[:, u * Fl:(u + 1) * Fl]
                                if blk_nops[b] == 0:
                                    empties.append(u)
                                    continue
                                o0 = int(blk_first[b])
                                nops = int(blk_nops[b])
                                for k in range(nops):
                                    o = o0 + k
                                    t, _b = ops[o]
                                    ch = int(chunk_of_tile[t])
                                    ensure_chunk(ch)
                                    # keep 2 chunks of gather in flight ahead
                                    for la in (1, 2):
                                        if ch + la < n_chunks:
                                            ensure_chunk(ch + la)
                                    S = spl.tile([128, 128], bf16, tag="S")
                                    eng = nc.vector if (o % 10) < DVE_OF_10 \
                                        else nc.gpsimd
                                    eng.tensor_scalar(
                                        S[:], iota_t[:], sl_sb[:, o:o + 1],
                                        iv_sb[:, o:o + 1],
                                        mybir.AluOpType.is_equal,
                                        mybir.AluOpType.mult)
                                    nc.tensor.matmul(
                                        sl_ps, S[:],
                                        msgs[ch][0][:, t - msgs[ch][1],
                                                     0:Fl],
                                        start=(k == 0), stop=(k == nops - 1))
                            # evacuate the whole slab in one ACT op
                            if len(empties) < w:
                                nc.scalar.activation(
                                    stg_t[:, jj:jj + w, :].rearrange(
                                        "p g c -> p (g c)"),
                                    ps[:, 0:w * Fl],
                                    mybir.ActivationFunctionType.Identity,
                                    bias=0.0, scale=1.0)
                            for u in empties:
                                nc.vector.memset(stg_t[:, jj + u, :], 0.0)
                            jj += w
                        nc.sync.dma_start(
                            parts[l][g][c * 128:(c + 1) * 128, :],
                            stg_t[:].rearrange("p g c -> p (g c)"))
                    nc.gpsimd.collective_compute(
                        "ReduceScatter", mybir.AluOpType.add,
                        replica_groups=[list(range(NCORES))],
                        ins=[parts[l][g][:].opt()],
                        outs=[agg_d[l][g][:].opt()])
                for g in range(len(SEGS)):
                    if g not in emitted:
                        emit_epilogue(g)

    nc.compile()
    return nc


def kernel(features, src, dst, W0, b0, W1, b1, W2, b2):
    features = np.asarray(features, dtype=np.float32)
    src = np.asarray(src).astype(np.int64)
    dst = np.asarray(dst).astype(np.int64)
    in_maps, sched = _prep(features, src, dst,
                           np.asarray(W0), np.asarray(b0), np.asarray(W1),
                           np.asarray(b1), np.asarray(W2), np.asarray(b2))
    key = (sched["T"], sched["NOPS"],
           hash(sched["blk_first"].tobytes()),
           hash(sched["blk_nops"].tobytes()))
    if _cache.get("key") != key:
        _cache["nc"] = _build(sched)
        _cache["key"] = key
    nc = _cache["nc"]
    res = run_bass_kernel_spmd(nc, in_maps, core_ids=list(range(NCORES)))
    full = np.concatenate([np.asarray(res.results[c]["out"]).astype(np.float32)
                           for c in range(NCORES)], axis=0)
    # rows are in permuted (balanced-block) order; node n sits at permn[n]
    return np.ascontiguousarray(full[sched["permn"][:N_NODES], :N_CLASSES])
